# revision 27
# baseline (speedup 1.0000x reference)
"""Trainium2 Bass kernel for nn_DecoderStack (cross-attention decoder stack).

Strategy (v2): single NeuronCore, zero collectives, weights baked into the
NEFF as bf16 constants.

Why: on this axon-tunneled platform the per-call cost is dominated by
fixed dispatch overhead (~4-5 ms/device-mesh) plus re-staging of every
ExternalInput buffer (~11 GB/s). The old 8-core tensor-parallel kernel
shipped 341 MB of fp32 weights per call => ~40 ms/call. Device compute for
the whole model is only ~1.5 ms on one core in bf16. So:
  - run on ONE core (smallest dispatch floor),
  - bake all weights into the NEFF via inline_tensor (Const tensors are
    loaded to HBM once at model-load, not per call),
  - per-call ExternalInputs are just the activations (~8 MB, all bf16):
    x0, encoder^T, and the precomputed time-bias qs.

The decoder rows are fully independent (enc-dec cross attention only; the
self-attn inputs are unused by the reference), so all B*F=1024 rows stream
through the stack with no inter-row dependencies. All matmuls run in bf16
(fp32 PSUM accumulate); layernorm/softmax statistics stay fp32.

kernel() re-builds (and re-compiles) the program if called with weights
whose bytes differ from the baked ones - correctness never depends on the
cache, only speed.
"""
import hashlib
import numpy as np
from contextlib import ExitStack

import concourse.bass as bass
import concourse.bacc as bacc
import concourse.tile as tile
from concourse import mybir

B, F, T = 2, 512, 512
D, N, H = 1024, 16, 64
KSZ = 32
FILT = 4096
L = 4
EPS = 1e-6

ROWS = B * F          # 1024 decoder rows total
RT = ROWS // 128      # 8 row-tiles
NB = B                # batches on this core
DC = D // 128         # 8 contraction chunks
FK = FILT // 128      # 32 filter chunks
HC = N * H // 128     # 8 head-dim chunks (2 heads per chunk)

FP = mybir.dt.float32
BF = mybir.dt.bfloat16
AF = mybir.ActivationFunctionType
OP = mybir.AluOpType
AX = mybir.AxisListType

BF_NP = mybir.dt.np(BF)


# ---------------------------------------------------------------- host prep

def _prep_consts(inputs):
    """Weight-derived constant arrays (baked into the NEFF, bf16)."""
    Wq = (np.asarray(inputs["Wq"], np.float32) * np.float32(H ** -0.5)
          ).reshape(L, D, N * H)
    Wk = np.asarray(inputs["Wk"], np.float32).reshape(L, D, N * H)
    Wv = np.asarray(inputs["Wv"], np.float32).reshape(L, D, N * H)
    Wo = np.asarray(inputs["Wo"], np.float32).reshape(L, N * H, D)
    Wf1 = np.asarray(inputs["Wf1"], np.float32)            # [L, D, FILT]
    bf1 = np.asarray(inputs["bf1"], np.float32)            # [L, FILT]
    Wf2 = np.asarray(inputs["Wf2"], np.float32)            # [L, FILT, D]
    bf2 = np.asarray(inputs["bf2"], np.float32)            # [L, D]

    # wf1 pre-tiled for lhsT streaming: [L, FK, DC, 128, 128]
    wf1_t = Wf1.reshape(L, DC, 128, FK, 128).transpose(0, 3, 1, 2, 4)

    consts = {
        "wq": Wq.astype(BF_NP),
        "wk": Wk.astype(BF_NP),
        "wv": Wv.astype(BF_NP),
        "wo": Wo.astype(BF_NP),
        "wf1t": np.ascontiguousarray(wf1_t).astype(BF_NP),
        "wf2": Wf2.astype(BF_NP),
        "bf1": bf1.reshape(L, FK, 128).astype(BF_NP),      # [L, FK, 128]
        "bf2": bf2.reshape(L, 1, D).astype(BF_NP),
        "idm": np.eye(128, dtype=np.float32).astype(BF_NP),
        "ones": np.ones((1, 128), np.float32).astype(BF_NP),
    }
    return consts


def _prep_call_inputs(inputs):
    """Per-call activation inputs (ExternalInput, small)."""
    di = np.asarray(inputs["decoder_inputs"], np.float32)
    eo = np.asarray(inputs["encoder_outputs"], np.float32)
    dist = np.asarray(inputs["decoder_encoder_times_dist"], np.float32)
    eb = np.asarray(inputs["enc_dec_attn_bias"], np.float32)
    Wth = np.asarray(inputs["Wth"], np.float32)
    bth = np.asarray(inputs["bth"], np.float32)
    Wto = np.asarray(inputs["Wto"], np.float32)
    bto = np.asarray(inputs["bto"], np.float32)

    # exact time-bias qs[i,b,f,t] = relu(d*Wth+bth) @ Wto + bto + eb[b,t]
    qs = np.empty((L, B, F, T), np.float32)
    for i in range(L):
        for f0 in range(0, F, 64):   # chunked: keep the [.,64,T,K] temp small
            h = np.maximum(dist[:, f0:f0 + 64, :, None] * Wth[i, 0] + bth[i],
                           0.0)
            qs[i, :, f0:f0 + 64] = h @ Wto[i, :, 0] + bto[i, 0]
    qs += eb[:, 0, 0][:, None, :][None]

    return {
        "x0": np.ascontiguousarray(di.reshape(ROWS, D)).astype(BF_NP),
        "encT": np.ascontiguousarray(eo.transpose(0, 2, 1)).astype(BF_NP),
        "qs": np.ascontiguousarray(qs.reshape(L, ROWS, T)).astype(BF_NP),
    }


def _weights_key(inputs):
    hsh = hashlib.sha256()
    for k in ("Wq", "Wk", "Wv", "Wo", "Wf1", "bf1", "Wf2", "bf2"):
        hsh.update(np.ascontiguousarray(np.asarray(inputs[k], np.float32)))
    return hsh.hexdigest()


# ------------------------------------------------------------ device program

def build_program(consts):
    nc = bacc.Bacc("TRN2", target_bir_lowering=False, debug=False,
                   num_devices=1)

    x0_d = nc.dram_tensor("x0", [ROWS, D], BF, kind="ExternalInput")
    encT_d = nc.dram_tensor("encT", [NB, D, T], BF, kind="ExternalInput")
    qs_d = nc.dram_tensor("qs", [L, ROWS, T], BF, kind="ExternalInput")
    yout_d = nc.dram_tensor("yout", [ROWS, D], BF, kind="ExternalOutput")

    wq_d = nc.inline_tensor(consts["wq"], name="wq")
    wk_d = nc.inline_tensor(consts["wk"], name="wk")
    wv_d = nc.inline_tensor(consts["wv"], name="wv")
    wo_d = nc.inline_tensor(consts["wo"], name="wo")
    wf1_d = nc.inline_tensor(consts["wf1t"], name="wf1t")
    wf2_d = nc.inline_tensor(consts["wf2"], name="wf2")
    bf1_d = nc.inline_tensor(consts["bf1"], name="bf1")
    bf2_d = nc.inline_tensor(consts["bf2"], name="bf2")
    id_d = nc.inline_tensor(consts["idm"], name="idm")
    ones_d = nc.inline_tensor(consts["ones"], name="ones")

    with tile.TileContext(nc) as tc, ExitStack() as ctx:
        per = ctx.enter_context(tc.tile_pool(name="per", bufs=1))
        kvp = ctx.enter_context(tc.tile_pool(name="kvp", bufs=1))
        wkv_p = ctx.enter_context(tc.tile_pool(name="wkv", bufs=2))
        wqo_p = ctx.enter_context(tc.tile_pool(name="wqo", bufs=1))
        lnp = ctx.enter_context(tc.tile_pool(name="lnp", bufs=2))
        blkp = ctx.enter_context(tc.tile_pool(name="blkp", bufs=1))
        qsp = ctx.enter_context(tc.tile_pool(name="qsp", bufs=2))
        smp = ctx.enter_context(tc.tile_pool(name="smp", bufs=2))
        otp = ctx.enter_context(tc.tile_pool(name="otp", bufs=2))
        htp = ctx.enter_context(tc.tile_pool(name="htp", bufs=1))
        wfp = ctx.enter_context(tc.tile_pool(name="wfp", bufs=3))
        psA = ctx.enter_context(tc.tile_pool(name="psA", bufs=2, space="PSUM"))
        psB = ctx.enter_context(tc.tile_pool(name="psB", bufs=2, space="PSUM"))
        psC = ctx.enter_context(tc.tile_pool(name="psC", bufs=2, space="PSUM"))
        psD = ctx.enter_context(tc.tile_pool(name="psD", bufs=2, space="PSUM"))
        ps_all = [psA, psB, psC, psD]

        def ps8():
            """Grab all 8 PSUM banks as 8 [128,512] fp32 tiles."""
            return [ps_all[j // 2].tile([128, 512], FP, tag="ABCD"[j // 2],
                                        name=f"ps8_{j}")
                    for j in range(8)]

        # ---- persistent tiles
        x_sb = per.tile([128, RT * D], FP)          # residual stream (rows)
        id_sb = per.tile([128, 128], BF)
        ones_sb = per.tile([1, 128], BF)
        enc_sb = per.tile([128, NB * DC * T], BF)   # encT both batches

        nc.sync.dma_start(id_sb[:], id_d[:, :])
        nc.sync.dma_start(ones_sb[:], ones_d[:, :])
        for r in range(RT):
            x0b = lnp.tile([128, D], BF, tag="xn")
            nc.sync.dma_start(x0b[:], x0_d[r * 128:(r + 1) * 128, :])
            nc.vector.tensor_copy(x_sb[:, r * D:(r + 1) * D], x0b[:])
        for nb in range(NB):
            for dc in range(DC):
                nc.sync.dma_start(
                    enc_sb[:, (nb * DC + dc) * T:(nb * DC + dc + 1) * T],
                    encT_d[nb, dc * 128:(dc + 1) * 128, :])

        def layer_norm(src_ap, dst_tile):
            """dst = (src - mean)/(std+eps); dst may be bf16."""
            scr = lnp.tile([128, D], FP, tag="scr", bufs=1)
            s1 = lnp.tile([128, 1], FP, tag="s1")
            nc.vector.tensor_reduce(s1[:], src_ap, AX.X, OP.add)
            sq = lnp.tile([128, 1], FP, tag="sq")
            nc.vector.scalar_tensor_tensor(scr[:], src_ap, 0.0, src_ap,
                                           OP.add, OP.mult, accum_out=sq[:])
            mean = lnp.tile([128, 1], FP, tag="mean")
            nc.scalar.mul(mean[:], s1[:], 1.0 / D)
            msq = lnp.tile([128, 1], FP, tag="msq")
            nc.vector.tensor_tensor(msq[:], mean[:], mean[:], OP.mult)
            var = lnp.tile([128, 1], FP, tag="var")
            nc.vector.scalar_tensor_tensor(var[:], sq[:], 1.0 / D, msq[:],
                                           OP.mult, OP.subtract)
            sd = lnp.tile([128, 1], FP, tag="sd")
            nc.scalar.activation(sd[:], var[:], AF.Sqrt)
            sde = lnp.tile([128, 1], FP, tag="sde")
            nc.vector.tensor_scalar_add(sde[:], sd[:], EPS)
            r_ = lnp.tile([128, 1], FP, tag="r")
            nc.vector.reciprocal(r_[:], sde[:])
            nb_ = lnp.tile([128, 1], FP, tag="nb")
            nc.vector.scalar_tensor_tensor(nb_[:], mean[:], -1.0, r_[:],
                                           OP.mult, OP.mult)
            nc.scalar.activation(dst_tile, src_ap, AF.Identity,
                                 bias=nb_[:, :1], scale=r_[:, :1])

        def transpose_rows(xn_tile, dstT, rt):
            """xn [128,D] bf16 -> dstT [128, DC*512] slices at col rt*128."""
            for g in range(2):          # two groups of 4 D-chunks
                pt = psD.tile([128, 512], BF, tag="D")
                for j in range(4):
                    dc = g * 4 + j
                    nc.tensor.transpose(pt[:, j * 128:(j + 1) * 128],
                                        xn_tile[:, dc * 128:(dc + 1) * 128],
                                        id_sb[:])
                dst = dstT[:, :].rearrange("p (dc r) -> p dc r", r=512)
                nc.vector.tensor_copy(
                    dst[:, g * 4:(g + 1) * 4, rt * 128:(rt + 1) * 128],
                    pt[:].rearrange("p (j r) -> p j r", j=4))
            return dstT

        # kT/v tiles for both batches (per layer)
        kT_sb = kvp.tile([128, NB * HC * T], BF, tag="kT")
        v_sb = kvp.tile([128, NB * 4 * N * H], BF, tag="v")

        def kv_proj(i):
            """K^T and V for both batches; wk/wv streamed per D-chunk with
            all 8 PSUM banks live as accumulators."""
            for nb in range(NB):
                kps = ps8()
                for dc in range(DC):
                    wk_sb = wkv_p.tile([128, N * H], BF, tag="wkv")
                    nc.sync.dma_start(wk_sb[:],
                                      wk_d[i, dc * 128:(dc + 1) * 128, :])
                    for hc in range(HC):
                        nc.tensor.matmul(
                            kps[hc][:],
                            wk_sb[:, hc * 128:(hc + 1) * 128],
                            enc_sb[:, (nb * DC + dc) * T:(nb * DC + dc + 1) * T],
                            start=(dc == 0), stop=(dc == DC - 1))
                for hc in range(HC):
                    nc.vector.tensor_copy(
                        kT_sb[:, (nb * HC + hc) * T:(nb * HC + hc + 1) * T],
                        kps[hc][:])
                vps = ps8()
                for dc in range(DC):
                    wv_sb = wkv_p.tile([128, N * H], BF, tag="wkv")
                    nc.sync.dma_start(wv_sb[:],
                                      wv_d[i, dc * 128:(dc + 1) * 128, :])
                    for tc in range(4):
                        for hh in range(2):
                            nc.tensor.matmul(
                                vps[tc * 2 + hh][:],
                                enc_sb[:, (nb * DC + dc) * T + tc * 128:
                                       (nb * DC + dc) * T + (tc + 1) * 128],
                                wv_sb[:, hh * 512:(hh + 1) * 512],
                                start=(dc == 0), stop=(dc == DC - 1))
                for tc in range(4):
                    for hh in range(2):
                        nc.vector.tensor_copy(
                            v_sb[:, (nb * 4 + tc) * N * H + hh * 512:
                                 (nb * 4 + tc) * N * H + (hh + 1) * 512],
                            vps[tc * 2 + hh][:])

        for i in range(L):
            # ---------------- K/V projections (both batches) ----------------
            kv_proj(i)

            # ---------------- attention ----------------
            wo_sb = wqo_p.tile([128, HC * D], BF, tag="wo")
            for hc in range(HC):
                nc.sync.dma_start(
                    wo_sb[:, hc * D:(hc + 1) * D],
                    wo_d[i, hc * 128:(hc + 1) * 128, :])

            for blk in range(RT // 4):        # 512-row blocks (one batch each)
                xnT = blkp.tile([128, DC * 512], BF, tag="xnT", bufs=2)
                for rt in range(4):
                    r = blk * 4 + rt
                    xn = lnp.tile([128, D], BF, tag="xn")
                    layer_norm(x_sb[:, r * D:(r + 1) * D], xn[:])
                    transpose_rows(xn, xnT, rt)

                # Q^T for the whole block: [h-chunk, 512 rows]; wq streamed
                qT = blkp.tile([128, HC * 512], BF, tag="qT", bufs=1)
                qps = ps8()
                for dc in range(DC):
                    wq_sb = wkv_p.tile([128, N * H], BF, tag="wkv")
                    nc.sync.dma_start(wq_sb[:],
                                      wq_d[i, dc * 128:(dc + 1) * 128, :])
                    for hc in range(HC):
                        nc.tensor.matmul(
                            qps[hc][:],
                            wq_sb[:, hc * 128:(hc + 1) * 128],
                            xnT[:, dc * 512:(dc + 1) * 512],
                            start=(dc == 0), stop=(dc == DC - 1))
                for hc in range(HC):
                    nc.vector.tensor_copy(qT[:, hc * 512:(hc + 1) * 512],
                                          qps[hc][:])

                for rt in range(4):
                    r = blk * 4 + rt
                    qs_sb = qsp.tile([128, T], BF, tag="qs")
                    nc.sync.dma_start(qs_sb[:],
                                      qs_d[i, r * 128:(r + 1) * 128, :])
                    qsf = qsp.tile([128, T], FP, tag="qsf")
                    nc.vector.tensor_copy(qsf[:], qs_sb[:])
                    oT_pss = [psC.tile([128, 512], FP, tag="C",
                                       name=f"oTps_{half}")
                              for half in range(2)]
                    for n in range(N):
                        hc, hr = n // 2, (n % 2) * 64
                        # 4-deep logits pipeline: alternate psA/psB banks
                        lg = ps_all[n % 2].tile([128, T], FP, tag="AB"[n % 2],
                                                name=f"lg{n % 2}")
                        nc.tensor.matmul(
                            lg[:],
                            qT[hr:hr + 64, hc * 512 + rt * 128:
                               hc * 512 + (rt + 1) * 128],
                            kT_sb[hr:hr + 64, (blk * HC + hc) * T:
                                  (blk * HC + hc + 1) * T],
                            start=True, stop=True)
                        # bias add in place in PSUM (GPSIMD cannot read PSUM)
                        nc.vector.tensor_tensor(lg[:], lg[:], qsf[:], OP.add)
                        den = lnp.tile([128, 1], FP, tag="den")
                        we = smp.tile([128, T], BF, tag="we")
                        nc.scalar.activation(we[:], lg[:], AF.Exp,
                                             accum_out=den[:])
                        rec = lnp.tile([128, 1], FP, tag="rec")
                        nc.vector.reciprocal(rec[:], den[:])
                        ws = smp.tile([128, T], BF, tag="ws")
                        if n % 2 == 0:
                            nc.vector.tensor_scalar_mul(ws[:], we[:],
                                                        rec[:, :1])
                        else:
                            nc.scalar.mul(ws[:], we[:], rec[:, :1])
                        # transpose w -> [t, f] chunks (psD is idle here)
                        pt = psD.tile([128, 4 * 128], BF, tag="D")
                        for tc in range(4):
                            nc.tensor.transpose(
                                pt[:, tc * 128:(tc + 1) * 128],
                                ws[:, tc * 128:(tc + 1) * 128], id_sb[:])
                        wT = smp.tile([128, 4 * 128], BF, tag="wT")
                        nc.vector.tensor_copy(wT[:], pt[:])
                        # AV: accumulate into oT psum rows for this head
                        ps_o = oT_pss[hc // 4]
                        for tc in range(4):
                            nc.tensor.matmul(
                                ps_o[hr:hr + 64,
                                     (hc % 4) * 128:(hc % 4 + 1) * 128],
                                v_sb[:, (blk * 4 + tc) * N * H + n * 64:
                                     (blk * 4 + tc) * N * H + (n + 1) * 64],
                                wT[:, tc * 128:(tc + 1) * 128],
                                start=(tc == 0), stop=(tc == 3))
                    oT = otp.tile([128, HC * 128], BF, tag="oT")
                    for half in range(2):
                        nc.vector.tensor_copy(
                            oT[:, half * 512:(half + 1) * 512],
                            oT_pss[half][:])
                    # O-projection + residual add (straight from PSUM)
                    for dh in range(2):
                        ps = psD.tile([128, 512], FP, tag="D")
                        for hc in range(HC):
                            nc.tensor.matmul(
                                ps[:],
                                oT[:, hc * 128:(hc + 1) * 128],
                                wo_sb[:, hc * D + dh * 512:
                                      hc * D + (dh + 1) * 512],
                                start=(hc == 0), stop=(hc == HC - 1))
                        xs = x_sb[:, r * D + dh * 512:r * D + (dh + 1) * 512]
                        nc.vector.tensor_tensor(xs, xs, ps[:], OP.add)

            # ---------------- FFN ----------------
            bf1_sb = lnp.tile([128, FK], BF, tag="bf1", bufs=1)
            nc.sync.dma_start(bf1_sb[:],
                              bf1_d[i].rearrange("fk p -> p fk"))
            bf2_sb = lnp.tile([1, D], BF, tag="bf2", bufs=1)
            nc.sync.dma_start(bf2_sb[:], bf2_d[i])

            for blk in range(RT // 4):
                xnT = blkp.tile([128, DC * 512], BF, tag="xnT", bufs=2)
                for rt in range(4):
                    r = blk * 4 + rt
                    xn = lnp.tile([128, D], BF, tag="xn")
                    layer_norm(x_sb[:, r * D:(r + 1) * D], xn[:])
                    transpose_rows(xn, xnT, rt)

                # FFN1: hT[filt-chunk, 512 rows] with fused bias+relu
                hT = htp.tile([128, FK * 512], BF, tag="hT")
                for fk in range(FK):
                    wf1_sb = wfp.tile([128, DC * 128], BF, tag="wf1")
                    nc.sync.dma_start(
                        wf1_sb[:].rearrange("p (dc c) -> p dc c", dc=DC),
                        wf1_d[i, fk].rearrange("dc p c -> p dc c"))
                    ps = ps_all[fk % 2].tile([128, 512], FP, tag="AB"[fk % 2])
                    for dc in range(DC):
                        nc.tensor.matmul(
                            ps[:],
                            wf1_sb[:, dc * 128:(dc + 1) * 128],
                            xnT[:, dc * 512:(dc + 1) * 512],
                            start=(dc == 0), stop=(dc == DC - 1))
                    nc.scalar.activation(hT[:, fk * 512:(fk + 1) * 512],
                                         ps[:], AF.Relu,
                                         bias=bf1_sb[:, fk:fk + 1])

                # FFN2: y2[f, D] accumulated over all filter chunks
                y2_ps = ps8()
                for fk in range(FK):
                    wf2_sb = wfp.tile([128, D], BF, tag="wf2")
                    nc.sync.dma_start(
                        wf2_sb[:], wf2_d[i, fk * 128:(fk + 1) * 128, :])
                    for rt in range(4):
                        for dh in range(2):
                            nc.tensor.matmul(
                                y2_ps[rt * 2 + dh][:],
                                hT[:, fk * 512 + rt * 128:
                                   fk * 512 + (rt + 1) * 128],
                                wf2_sb[:, dh * 512:(dh + 1) * 512],
                                start=(fk == 0), stop=False)
                # bias fold: y2 += ones^T @ bf2  (K=1 matmul, ends the group)
                for rt in range(4):
                    for dh in range(2):
                        nc.tensor.matmul(
                            y2_ps[rt * 2 + dh][:],
                            ones_sb[:, :],
                            bf2_sb[:, dh * 512:(dh + 1) * 512],
                            start=False, stop=True)
                for rt in range(4):
                    r = blk * 4 + rt
                    for dh in range(2):
                        xs = x_sb[:, r * D + dh * 512:r * D + (dh + 1) * 512]
                        nc.vector.tensor_tensor(xs, xs,
                                                y2_ps[rt * 2 + dh][:], OP.add)

        # final norm
        for r in range(RT):
            xf = lnp.tile([128, D], BF, tag="xfin")
            layer_norm(x_sb[:, r * D:(r + 1) * D], xf[:])
            nc.sync.dma_start(yout_d[r * 128:(r + 1) * 128, :], xf[:])

    nc.compile()
    return nc


# ------------------------------------------------------------ runner

_CACHE = {}   # weights_key -> (sharded, in_names, zero_outs)


def _build_runner(consts):
    import jax
    from jax.sharding import Mesh, PartitionSpec
    from jax.experimental.shard_map import shard_map
    from concourse import bass2jax

    nc = build_program(consts)
    partition_name = (nc.partition_id_tensor.name
                      if nc.partition_id_tensor else None)
    in_names, out_names, out_avals = [], [], []
    for alloc in nc.m.functions[0].allocations:
        if not isinstance(alloc, mybir.MemoryLocationSet):
            continue
        name = alloc.memorylocations[0].name
        if alloc.kind == "ExternalInput":
            if name != partition_name:
                in_names.append(name)
        elif alloc.kind == "ExternalOutput":
            out_names.append(name)
            out_avals.append(jax.core.ShapedArray(
                tuple(alloc.tensor_shape), mybir.dt.np(alloc.dtype)))
    all_names = in_names + out_names
    if partition_name is not None:
        all_names = all_names + [partition_name]

    def _body(*args):
        operands = list(args)
        if partition_name is not None:
            operands.append(bass2jax.partition_id_tensor())
        outs = bass2jax._bass_exec_p.bind(
            *operands,
            out_avals=tuple(out_avals),
            in_names=tuple(all_names),
            out_names=tuple(out_names),
            lowering_input_output_aliases=(),
            sim_require_finite=True,
            sim_require_nnan=True,
            nc=nc,
        )
        return tuple(outs)

    bass2jax.install_neuronx_cc_hook()
    devices = jax.devices()[:1]
    mesh = Mesh(np.asarray(devices), ("core",))
    n_all = len(in_names) + len(out_names)
    sharded = jax.jit(
        shard_map(_body, mesh=mesh,
                  in_specs=(PartitionSpec("core"),) * n_all,
                  out_specs=(PartitionSpec("core"),) * len(out_names),
                  check_rep=False),
        keep_unused=True,
    )
    zero_outs = [np.zeros(tuple(a.shape), a.dtype) for a in out_avals]
    return sharded, in_names, out_names, zero_outs


def _get_runner(inputs):
    key = _weights_key(inputs)
    if key not in _CACHE:
        consts = _prep_consts(inputs)
        _CACHE[key] = _build_runner(consts)
    return _CACHE[key]


def kernel(**inputs) -> np.ndarray:
    sharded, in_names, out_names, zero_outs = _get_runner(inputs)
    call_in = _prep_call_inputs(inputs)
    args = [call_in[nm] for nm in in_names] + zero_outs
    out_arrs = sharded(*args)
    yi = out_names.index("yout")
    yfull = np.asarray(out_arrs[yi]).reshape(B, F, D)
    return yfull


if __name__ == "__main__":
    import sys
    sys.path.insert(0, "/root/problem")
    import reference
    inputs = {k: np.asarray(v) for k, v in reference.setup_inputs().items()}
    expected = np.asarray(reference.reference(**inputs))
    got = kernel(**inputs)
    err = np.abs(got - expected).max() / np.abs(expected).max()
    print("rel err (absmax):", err)
    print("rel l2:", np.linalg.norm(got - expected) / np.linalg.norm(expected))


# revision 28
# speedup vs baseline: 1.3353x; 1.3353x over previous
"""Trainium2 Bass kernel for nn_DecoderStack (cross-attention decoder stack).

Strategy (v2): single NeuronCore, zero collectives, weights baked into the
NEFF as bf16 constants.

Why: on this axon-tunneled platform the per-call cost is dominated by
fixed dispatch overhead (~4-5 ms/device-mesh) plus re-staging of every
ExternalInput buffer (~11 GB/s). The old 8-core tensor-parallel kernel
shipped 341 MB of fp32 weights per call => ~40 ms/call. Device compute for
the whole model is only ~1.5 ms on one core in bf16. So:
  - run on ONE core (smallest dispatch floor),
  - bake all weights into the NEFF via inline_tensor (Const tensors are
    loaded to HBM once at model-load, not per call),
  - per-call ExternalInputs are just the activations (~8 MB, all bf16):
    x0, encoder^T, and the precomputed time-bias qs.

The decoder rows are fully independent (enc-dec cross attention only; the
self-attn inputs are unused by the reference), so all B*F=1024 rows stream
through the stack with no inter-row dependencies. All matmuls run in bf16
(fp32 PSUM accumulate); layernorm/softmax statistics stay fp32.

kernel() re-builds (and re-compiles) the program if called with weights
whose bytes differ from the baked ones - correctness never depends on the
cache, only speed.
"""
import hashlib
import numpy as np
from contextlib import ExitStack

import concourse.bass as bass
import concourse.bacc as bacc
import concourse.tile as tile
from concourse import mybir

B, F, T = 2, 512, 512
D, N, H = 1024, 16, 64
KSZ = 32
FILT = 4096
L = 4
EPS = 1e-6

ROWS = B * F          # 1024 decoder rows total
RT = ROWS // 128      # 8 row-tiles
NB = B                # batches on this core
DC = D // 128         # 8 contraction chunks
FK = FILT // 128      # 32 filter chunks
HC = N * H // 128     # 8 head-dim chunks (2 heads per chunk)

FP = mybir.dt.float32
BF = mybir.dt.bfloat16
AF = mybir.ActivationFunctionType
OP = mybir.AluOpType
AX = mybir.AxisListType

BF_NP = mybir.dt.np(BF)


# ---------------------------------------------------------------- host prep

def _prep_consts(inputs):
    """Weight-derived constant arrays (baked into the NEFF, bf16)."""
    Wq = (np.asarray(inputs["Wq"], np.float32) * np.float32(H ** -0.5)
          ).reshape(L, D, N * H)
    Wk = np.asarray(inputs["Wk"], np.float32).reshape(L, D, N * H)
    Wv = np.asarray(inputs["Wv"], np.float32).reshape(L, D, N * H)
    Wo = np.asarray(inputs["Wo"], np.float32).reshape(L, N * H, D)
    Wf1 = np.asarray(inputs["Wf1"], np.float32)            # [L, D, FILT]
    bf1 = np.asarray(inputs["bf1"], np.float32)            # [L, FILT]
    Wf2 = np.asarray(inputs["Wf2"], np.float32)            # [L, FILT, D]
    bf2 = np.asarray(inputs["bf2"], np.float32)            # [L, D]

    # wf1 pre-tiled for lhsT streaming: [L, FK, DC, 128, 128]
    wf1_t = Wf1.reshape(L, DC, 128, FK, 128).transpose(0, 3, 1, 2, 4)

    consts = {
        "wq": Wq.astype(BF_NP),
        "wk": Wk.astype(BF_NP),
        "wv": Wv.astype(BF_NP),
        "wo": Wo.astype(BF_NP),
        "wf1t": np.ascontiguousarray(wf1_t).astype(BF_NP),
        "wf2": Wf2.astype(BF_NP),
        "bf1": bf1.reshape(L, FK, 128).astype(BF_NP),      # [L, FK, 128]
        "bf2": bf2.reshape(L, 1, D).astype(BF_NP),
        "idm": np.eye(128, dtype=np.float32).astype(BF_NP),
        "ones": np.ones((1, 128), np.float32).astype(BF_NP),
    }
    return consts


def _prep_call_inputs(inputs):
    """Per-call activation inputs (ExternalInput, small)."""
    di = np.asarray(inputs["decoder_inputs"], np.float32)
    eo = np.asarray(inputs["encoder_outputs"], np.float32)
    dist = np.asarray(inputs["decoder_encoder_times_dist"], np.float32)
    eb = np.asarray(inputs["enc_dec_attn_bias"], np.float32)
    Wth = np.asarray(inputs["Wth"], np.float32)
    bth = np.asarray(inputs["bth"], np.float32)
    Wto = np.asarray(inputs["Wto"], np.float32)
    bto = np.asarray(inputs["bto"], np.float32)

    # exact time-bias qs[i,b,f,t] = relu(d*Wth+bth) @ Wto + bto + eb[b,t]
    qs = np.empty((L, B, F, T), np.float32)
    for i in range(L):
        for f0 in range(0, F, 64):   # chunked: keep the [.,64,T,K] temp small
            h = np.maximum(dist[:, f0:f0 + 64, :, None] * Wth[i, 0] + bth[i],
                           0.0)
            qs[i, :, f0:f0 + 64] = h @ Wto[i, :, 0] + bto[i, 0]
    qs += eb[:, 0, 0][:, None, :][None]

    return {
        "x0": np.ascontiguousarray(di.reshape(ROWS, D)).astype(BF_NP),
        "encT": np.ascontiguousarray(eo.transpose(0, 2, 1)).astype(BF_NP),
        "qs": np.ascontiguousarray(qs.reshape(L, ROWS, T)).astype(BF_NP),
    }


def _weights_key(inputs):
    hsh = hashlib.sha256()
    for k in ("Wq", "Wk", "Wv", "Wo", "Wf1", "bf1", "Wf2", "bf2"):
        hsh.update(np.ascontiguousarray(np.asarray(inputs[k], np.float32)))
    return hsh.hexdigest()


# ------------------------------------------------------------ device program

def build_program(consts):
    nc = bacc.Bacc("TRN2", target_bir_lowering=False, debug=False,
                   num_devices=1)

    x0_d = nc.dram_tensor("x0", [ROWS, D], BF, kind="ExternalInput")
    encT_d = nc.dram_tensor("encT", [NB, D, T], BF, kind="ExternalInput")
    qs_d = nc.dram_tensor("qs", [L, ROWS, T], BF, kind="ExternalInput")
    yout_d = nc.dram_tensor("yout", [ROWS, D], BF, kind="ExternalOutput")

    wq_d = nc.inline_tensor(consts["wq"], name="wq")
    wk_d = nc.inline_tensor(consts["wk"], name="wk")
    wv_d = nc.inline_tensor(consts["wv"], name="wv")
    wo_d = nc.inline_tensor(consts["wo"], name="wo")
    wf1_d = nc.inline_tensor(consts["wf1t"], name="wf1t")
    wf2_d = nc.inline_tensor(consts["wf2"], name="wf2")
    bf1_d = nc.inline_tensor(consts["bf1"], name="bf1")
    bf2_d = nc.inline_tensor(consts["bf2"], name="bf2")
    id_d = nc.inline_tensor(consts["idm"], name="idm")
    ones_d = nc.inline_tensor(consts["ones"], name="ones")

    with tile.TileContext(nc) as tc, ExitStack() as ctx:
        per = ctx.enter_context(tc.tile_pool(name="per", bufs=1))
        kvp = ctx.enter_context(tc.tile_pool(name="kvp", bufs=1))
        wkv_p = ctx.enter_context(tc.tile_pool(name="wkv", bufs=2))
        wqo_p = ctx.enter_context(tc.tile_pool(name="wqo", bufs=1))
        lnp = ctx.enter_context(tc.tile_pool(name="lnp", bufs=2))
        blkp = ctx.enter_context(tc.tile_pool(name="blkp", bufs=1))
        qsp = ctx.enter_context(tc.tile_pool(name="qsp", bufs=2))
        smp = ctx.enter_context(tc.tile_pool(name="smp", bufs=2))
        otp = ctx.enter_context(tc.tile_pool(name="otp", bufs=2))
        htp = ctx.enter_context(tc.tile_pool(name="htp", bufs=1))
        wfp = ctx.enter_context(tc.tile_pool(name="wfp", bufs=3))
        psA = ctx.enter_context(tc.tile_pool(name="psA", bufs=2, space="PSUM"))
        psB = ctx.enter_context(tc.tile_pool(name="psB", bufs=2, space="PSUM"))
        psC = ctx.enter_context(tc.tile_pool(name="psC", bufs=2, space="PSUM"))
        psD = ctx.enter_context(tc.tile_pool(name="psD", bufs=2, space="PSUM"))
        ps_all = [psA, psB, psC, psD]

        def ps8():
            """Grab all 8 PSUM banks as 8 [128,512] fp32 tiles."""
            return [ps_all[j // 2].tile([128, 512], FP, tag="ABCD"[j // 2],
                                        name=f"ps8_{j}")
                    for j in range(8)]

        # ---- persistent tiles
        x_sb = per.tile([128, RT * D], FP)          # residual stream (rows)
        id_sb = per.tile([128, 128], BF)
        ones_sb = per.tile([1, 128], BF)
        enc_sb = per.tile([128, NB * DC * T], BF)   # encT both batches

        nc.sync.dma_start(id_sb[:], id_d[:, :])
        nc.sync.dma_start(ones_sb[:], ones_d[:, :])
        for r in range(RT):
            x0b = lnp.tile([128, D], BF, tag="xn")
            nc.sync.dma_start(x0b[:], x0_d[r * 128:(r + 1) * 128, :])
            nc.vector.tensor_copy(x_sb[:, r * D:(r + 1) * D], x0b[:])
        for nb in range(NB):
            for dc in range(DC):
                nc.sync.dma_start(
                    enc_sb[:, (nb * DC + dc) * T:(nb * DC + dc + 1) * T],
                    encT_d[nb, dc * 128:(dc + 1) * 128, :])

        def layer_norm(src_ap, dst_tile):
            """dst = (src - mean)/(std+eps); dst may be bf16."""
            scr = lnp.tile([128, D], FP, tag="scr", bufs=1)
            s1 = lnp.tile([128, 1], FP, tag="s1")
            nc.vector.tensor_reduce(s1[:], src_ap, AX.X, OP.add)
            sq = lnp.tile([128, 1], FP, tag="sq")
            nc.vector.scalar_tensor_tensor(scr[:], src_ap, 0.0, src_ap,
                                           OP.add, OP.mult, accum_out=sq[:])
            mean = lnp.tile([128, 1], FP, tag="mean")
            nc.scalar.mul(mean[:], s1[:], 1.0 / D)
            msq = lnp.tile([128, 1], FP, tag="msq")
            nc.vector.tensor_tensor(msq[:], mean[:], mean[:], OP.mult)
            var = lnp.tile([128, 1], FP, tag="var")
            nc.vector.scalar_tensor_tensor(var[:], sq[:], 1.0 / D, msq[:],
                                           OP.mult, OP.subtract)
            sd = lnp.tile([128, 1], FP, tag="sd")
            nc.scalar.activation(sd[:], var[:], AF.Sqrt)
            sde = lnp.tile([128, 1], FP, tag="sde")
            nc.vector.tensor_scalar_add(sde[:], sd[:], EPS)
            r_ = lnp.tile([128, 1], FP, tag="r")
            nc.vector.reciprocal(r_[:], sde[:])
            nb_ = lnp.tile([128, 1], FP, tag="nb")
            nc.vector.scalar_tensor_tensor(nb_[:], mean[:], -1.0, r_[:],
                                           OP.mult, OP.mult)
            nc.scalar.activation(dst_tile, src_ap, AF.Identity,
                                 bias=nb_[:, :1], scale=r_[:, :1])

        def transpose_rows(xn_tile, dstT, rt):
            """xn [128,D] bf16 -> dstT [128, DC*512] slices at col rt*128."""
            for g in range(2):          # two groups of 4 D-chunks
                pt = psD.tile([128, 512], BF, tag="D")
                for j in range(4):
                    dc = g * 4 + j
                    nc.tensor.transpose(pt[:, j * 128:(j + 1) * 128],
                                        xn_tile[:, dc * 128:(dc + 1) * 128],
                                        id_sb[:])
                dst = dstT[:, :].rearrange("p (dc r) -> p dc r", r=512)
                nc.vector.tensor_copy(
                    dst[:, g * 4:(g + 1) * 4, rt * 128:(rt + 1) * 128],
                    pt[:].rearrange("p (j r) -> p j r", j=4))
            return dstT

        # kT/v tiles for both batches (per layer)
        kT_sb = kvp.tile([128, NB * HC * T], BF, tag="kT")
        v_sb = kvp.tile([128, NB * 4 * N * H], BF, tag="v")

        def kv_proj(i):
            """K^T and V for both batches; wk/wv streamed per D-chunk with
            all 8 PSUM banks live as accumulators."""
            for nb in range(NB):
                kps = ps8()
                for dc in range(DC):
                    wk_sb = wkv_p.tile([128, N * H], BF, tag="wkv")
                    nc.sync.dma_start(wk_sb[:],
                                      wk_d[i, dc * 128:(dc + 1) * 128, :])
                    for hc in range(HC):
                        nc.tensor.matmul(
                            kps[hc][:],
                            wk_sb[:, hc * 128:(hc + 1) * 128],
                            enc_sb[:, (nb * DC + dc) * T:(nb * DC + dc + 1) * T],
                            start=(dc == 0), stop=(dc == DC - 1))
                for hc in range(HC):
                    nc.vector.tensor_copy(
                        kT_sb[:, (nb * HC + hc) * T:(nb * HC + hc + 1) * T],
                        kps[hc][:])
                vps = ps8()
                for dc in range(DC):
                    wv_sb = wkv_p.tile([128, N * H], BF, tag="wkv")
                    nc.sync.dma_start(wv_sb[:],
                                      wv_d[i, dc * 128:(dc + 1) * 128, :])
                    for tc in range(4):
                        for hh in range(2):
                            nc.tensor.matmul(
                                vps[tc * 2 + hh][:],
                                enc_sb[:, (nb * DC + dc) * T + tc * 128:
                                       (nb * DC + dc) * T + (tc + 1) * 128],
                                wv_sb[:, hh * 512:(hh + 1) * 512],
                                start=(dc == 0), stop=(dc == DC - 1))
                for tc in range(4):
                    for hh in range(2):
                        nc.vector.tensor_copy(
                            v_sb[:, (nb * 4 + tc) * N * H + hh * 512:
                                 (nb * 4 + tc) * N * H + (hh + 1) * 512],
                            vps[tc * 2 + hh][:])

        for i in range(L):
            # ---------------- K/V projections (both batches) ----------------
            kv_proj(i)

            # ---------------- attention ----------------
            wo_sb = wqo_p.tile([128, HC * D], BF, tag="wo")
            for hc in range(HC):
                nc.sync.dma_start(
                    wo_sb[:, hc * D:(hc + 1) * D],
                    wo_d[i, hc * 128:(hc + 1) * 128, :])

            for blk in range(RT // 4):        # 512-row blocks (one batch each)
                xnT = blkp.tile([128, DC * 512], BF, tag="xnT", bufs=2)
                for rt in range(4):
                    r = blk * 4 + rt
                    xn = lnp.tile([128, D], BF, tag="xn")
                    layer_norm(x_sb[:, r * D:(r + 1) * D], xn[:])
                    transpose_rows(xn, xnT, rt)

                # Q^T for the whole block: [h-chunk, 512 rows]; wq streamed
                qT = blkp.tile([128, HC * 512], BF, tag="qT", bufs=1)
                qps = ps8()
                for dc in range(DC):
                    wq_sb = wkv_p.tile([128, N * H], BF, tag="wkv")
                    nc.sync.dma_start(wq_sb[:],
                                      wq_d[i, dc * 128:(dc + 1) * 128, :])
                    for hc in range(HC):
                        nc.tensor.matmul(
                            qps[hc][:],
                            wq_sb[:, hc * 128:(hc + 1) * 128],
                            xnT[:, dc * 512:(dc + 1) * 512],
                            start=(dc == 0), stop=(dc == DC - 1))
                for hc in range(HC):
                    nc.vector.tensor_copy(qT[:, hc * 512:(hc + 1) * 512],
                                          qps[hc][:])

                for rt in range(4):
                    r = blk * 4 + rt
                    qs_sb = qsp.tile([128, T], BF, tag="qs")
                    nc.sync.dma_start(qs_sb[:],
                                      qs_d[i, r * 128:(r + 1) * 128, :])
                    qsf = qsp.tile([128, T], FP, tag="qsf")
                    nc.vector.tensor_copy(qsf[:], qs_sb[:])
                    oT_pss = [psC.tile([128, 512], FP, tag="C",
                                       name=f"oTps_{half}")
                              for half in range(2)]
                    for n in range(N):
                        hc, hr = n // 2, (n % 2) * 64
                        # 4-deep logits pipeline: alternate psA/psB banks
                        lg = ps_all[n % 2].tile([128, T], FP, tag="AB"[n % 2],
                                                name=f"lg{n % 2}")
                        nc.tensor.matmul(
                            lg[:],
                            qT[hr:hr + 64, hc * 512 + rt * 128:
                               hc * 512 + (rt + 1) * 128],
                            kT_sb[hr:hr + 64, (blk * HC + hc) * T:
                                  (blk * HC + hc + 1) * T],
                            start=True, stop=True)
                        # bias add in place in PSUM (GPSIMD cannot read PSUM)
                        nc.vector.tensor_tensor(lg[:], lg[:], qsf[:], OP.add)
                        den = lnp.tile([128, 1], FP, tag="den", bufs=4)
                        we = smp.tile([128, T], BF, tag="we", bufs=3)
                        nc.scalar.activation(we[:], lg[:], AF.Exp,
                                             accum_out=den[:])
                        rec = lnp.tile([128, 1], FP, tag="rec", bufs=4)
                        nc.vector.reciprocal(rec[:], den[:])
                        ws = smp.tile([128, T], BF, tag="ws", bufs=3)
                        if n % 2 == 0:
                            nc.vector.tensor_scalar_mul(ws[:], we[:],
                                                        rec[:, :1])
                        else:
                            nc.scalar.mul(ws[:], we[:], rec[:, :1])
                        # transpose w -> [t, f] chunks (psD is idle here)
                        pt = psD.tile([128, 4 * 128], BF, tag="D")
                        for tc in range(4):
                            nc.tensor.transpose(
                                pt[:, tc * 128:(tc + 1) * 128],
                                ws[:, tc * 128:(tc + 1) * 128], id_sb[:])
                        wT = smp.tile([128, 4 * 128], BF, tag="wT", bufs=3)
                        nc.vector.tensor_copy(wT[:], pt[:])
                        # AV: accumulate into oT psum rows for this head
                        ps_o = oT_pss[hc // 4]
                        for tc in range(4):
                            nc.tensor.matmul(
                                ps_o[hr:hr + 64,
                                     (hc % 4) * 128:(hc % 4 + 1) * 128],
                                v_sb[:, (blk * 4 + tc) * N * H + n * 64:
                                     (blk * 4 + tc) * N * H + (n + 1) * 64],
                                wT[:, tc * 128:(tc + 1) * 128],
                                start=(tc == 0), stop=(tc == 3))
                    oT = otp.tile([128, HC * 128], BF, tag="oT")
                    for half in range(2):
                        nc.vector.tensor_copy(
                            oT[:, half * 512:(half + 1) * 512],
                            oT_pss[half][:])
                    # O-projection + residual add (straight from PSUM)
                    for dh in range(2):
                        ps = psD.tile([128, 512], FP, tag="D")
                        for hc in range(HC):
                            nc.tensor.matmul(
                                ps[:],
                                oT[:, hc * 128:(hc + 1) * 128],
                                wo_sb[:, hc * D + dh * 512:
                                      hc * D + (dh + 1) * 512],
                                start=(hc == 0), stop=(hc == HC - 1))
                        xs = x_sb[:, r * D + dh * 512:r * D + (dh + 1) * 512]
                        nc.vector.tensor_tensor(xs, xs, ps[:], OP.add)

            # ---------------- FFN ----------------
            bf1_sb = lnp.tile([128, FK], BF, tag="bf1", bufs=1)
            nc.sync.dma_start(bf1_sb[:],
                              bf1_d[i].rearrange("fk p -> p fk"))
            bf2_sb = lnp.tile([1, D], BF, tag="bf2", bufs=1)
            nc.sync.dma_start(bf2_sb[:], bf2_d[i])

            for blk in range(RT // 4):
                xnT = blkp.tile([128, DC * 512], BF, tag="xnT", bufs=2)
                for rt in range(4):
                    r = blk * 4 + rt
                    xn = lnp.tile([128, D], BF, tag="xn")
                    layer_norm(x_sb[:, r * D:(r + 1) * D], xn[:])
                    transpose_rows(xn, xnT, rt)

                # FFN1: hT[filt-chunk, 512 rows] with fused bias+relu
                hT = htp.tile([128, FK * 512], BF, tag="hT")
                for fk in range(FK):
                    wf1_sb = wfp.tile([128, DC * 128], BF, tag="wf1")
                    nc.sync.dma_start(
                        wf1_sb[:].rearrange("p (dc c) -> p dc c", dc=DC),
                        wf1_d[i, fk].rearrange("dc p c -> p dc c"))
                    ps = ps_all[fk % 2].tile([128, 512], FP, tag="AB"[fk % 2])
                    for dc in range(DC):
                        nc.tensor.matmul(
                            ps[:],
                            wf1_sb[:, dc * 128:(dc + 1) * 128],
                            xnT[:, dc * 512:(dc + 1) * 512],
                            start=(dc == 0), stop=(dc == DC - 1))
                    nc.scalar.activation(hT[:, fk * 512:(fk + 1) * 512],
                                         ps[:], AF.Relu,
                                         bias=bf1_sb[:, fk:fk + 1])

                # FFN2: y2[f, D] accumulated over all filter chunks
                y2_ps = ps8()
                for fk in range(FK):
                    wf2_sb = wfp.tile([128, D], BF, tag="wf2")
                    nc.sync.dma_start(
                        wf2_sb[:], wf2_d[i, fk * 128:(fk + 1) * 128, :])
                    for rt in range(4):
                        for dh in range(2):
                            nc.tensor.matmul(
                                y2_ps[rt * 2 + dh][:],
                                hT[:, fk * 512 + rt * 128:
                                   fk * 512 + (rt + 1) * 128],
                                wf2_sb[:, dh * 512:(dh + 1) * 512],
                                start=(fk == 0), stop=False)
                # bias fold: y2 += ones^T @ bf2  (K=1 matmul, ends the group)
                for rt in range(4):
                    for dh in range(2):
                        nc.tensor.matmul(
                            y2_ps[rt * 2 + dh][:],
                            ones_sb[:, :],
                            bf2_sb[:, dh * 512:(dh + 1) * 512],
                            start=False, stop=True)
                for rt in range(4):
                    r = blk * 4 + rt
                    for dh in range(2):
                        xs = x_sb[:, r * D + dh * 512:r * D + (dh + 1) * 512]
                        nc.vector.tensor_tensor(xs, xs,
                                                y2_ps[rt * 2 + dh][:], OP.add)

        # final norm
        for r in range(RT):
            xf = lnp.tile([128, D], BF, tag="xfin")
            layer_norm(x_sb[:, r * D:(r + 1) * D], xf[:])
            nc.sync.dma_start(yout_d[r * 128:(r + 1) * 128, :], xf[:])

    nc.compile()
    return nc


# ------------------------------------------------------------ runner

_CACHE = {}   # weights_key -> (sharded, in_names, zero_outs)


def _build_runner(consts):
    import jax
    from jax.sharding import Mesh, PartitionSpec
    from jax.experimental.shard_map import shard_map
    from concourse import bass2jax

    nc = build_program(consts)
    partition_name = (nc.partition_id_tensor.name
                      if nc.partition_id_tensor else None)
    in_names, out_names, out_avals = [], [], []
    for alloc in nc.m.functions[0].allocations:
        if not isinstance(alloc, mybir.MemoryLocationSet):
            continue
        name = alloc.memorylocations[0].name
        if alloc.kind == "ExternalInput":
            if name != partition_name:
                in_names.append(name)
        elif alloc.kind == "ExternalOutput":
            out_names.append(name)
            out_avals.append(jax.core.ShapedArray(
                tuple(alloc.tensor_shape), mybir.dt.np(alloc.dtype)))
    all_names = in_names + out_names
    if partition_name is not None:
        all_names = all_names + [partition_name]

    def _body(*args):
        operands = list(args)
        if partition_name is not None:
            operands.append(bass2jax.partition_id_tensor())
        outs = bass2jax._bass_exec_p.bind(
            *operands,
            out_avals=tuple(out_avals),
            in_names=tuple(all_names),
            out_names=tuple(out_names),
            lowering_input_output_aliases=(),
            sim_require_finite=True,
            sim_require_nnan=True,
            nc=nc,
        )
        return tuple(outs)

    bass2jax.install_neuronx_cc_hook()
    devices = jax.devices()[:1]
    mesh = Mesh(np.asarray(devices), ("core",))
    n_all = len(in_names) + len(out_names)
    sharded = jax.jit(
        shard_map(_body, mesh=mesh,
                  in_specs=(PartitionSpec("core"),) * n_all,
                  out_specs=(PartitionSpec("core"),) * len(out_names),
                  check_rep=False),
        keep_unused=True,
    )
    zero_outs = [np.zeros(tuple(a.shape), a.dtype) for a in out_avals]
    return sharded, in_names, out_names, zero_outs


def _get_runner(inputs):
    key = _weights_key(inputs)
    if key not in _CACHE:
        consts = _prep_consts(inputs)
        _CACHE[key] = _build_runner(consts)
    return _CACHE[key]


def kernel(**inputs) -> np.ndarray:
    sharded, in_names, out_names, zero_outs = _get_runner(inputs)
    call_in = _prep_call_inputs(inputs)
    args = [call_in[nm] for nm in in_names] + zero_outs
    out_arrs = sharded(*args)
    yi = out_names.index("yout")
    yfull = np.asarray(out_arrs[yi]).reshape(B, F, D)
    return yfull


if __name__ == "__main__":
    import sys
    sys.path.insert(0, "/root/problem")
    import reference
    inputs = {k: np.asarray(v) for k, v in reference.setup_inputs().items()}
    expected = np.asarray(reference.reference(**inputs))
    got = kernel(**inputs)
    err = np.abs(got - expected).max() / np.abs(expected).max()
    print("rel err (absmax):", err)
    print("rel l2:", np.linalg.norm(got - expected) / np.linalg.norm(expected))


# revision 31
# speedup vs baseline: 1.3522x; 1.0127x over previous
"""Trainium2 Bass kernel for nn_DecoderStack (cross-attention decoder stack).

Strategy (v2): single NeuronCore, zero collectives, weights baked into the
NEFF as bf16 constants.

Why: on this axon-tunneled platform the per-call cost is dominated by
fixed dispatch overhead (~4-5 ms/device-mesh) plus re-staging of every
ExternalInput buffer (~11 GB/s). The old 8-core tensor-parallel kernel
shipped 341 MB of fp32 weights per call => ~40 ms/call. Device compute for
the whole model is only ~1.5 ms on one core in bf16. So:
  - run on ONE core (smallest dispatch floor),
  - bake all weights into the NEFF via inline_tensor (Const tensors are
    loaded to HBM once at model-load, not per call),
  - per-call ExternalInputs are just the activations (~8 MB, all bf16):
    x0, encoder^T, and the precomputed time-bias qs.

The decoder rows are fully independent (enc-dec cross attention only; the
self-attn inputs are unused by the reference), so all B*F=1024 rows stream
through the stack with no inter-row dependencies. All matmuls run in bf16
(fp32 PSUM accumulate); layernorm/softmax statistics stay fp32.

kernel() re-builds (and re-compiles) the program if called with weights
whose bytes differ from the baked ones - correctness never depends on the
cache, only speed.
"""
import hashlib
import numpy as np
from contextlib import ExitStack

import concourse.bass as bass
import concourse.bacc as bacc
import concourse.tile as tile
from concourse import mybir

B, F, T = 2, 512, 512
D, N, H = 1024, 16, 64
KSZ = 32
FILT = 4096
L = 4
EPS = 1e-6

ROWS = B * F          # 1024 decoder rows total
RT = ROWS // 128      # 8 row-tiles
NB = B                # batches on this core
DC = D // 128         # 8 contraction chunks
FK = FILT // 128      # 32 filter chunks
HC = N * H // 128     # 8 head-dim chunks (2 heads per chunk)

FP = mybir.dt.float32
BF = mybir.dt.bfloat16
AF = mybir.ActivationFunctionType
OP = mybir.AluOpType
AX = mybir.AxisListType

BF_NP = mybir.dt.np(BF)


# ---------------------------------------------------------------- host prep

def _prep_consts(inputs):
    """Weight-derived constant arrays (baked into the NEFF, bf16)."""
    Wq = (np.asarray(inputs["Wq"], np.float32) * np.float32(H ** -0.5)
          ).reshape(L, D, N * H)
    Wk = np.asarray(inputs["Wk"], np.float32).reshape(L, D, N * H)
    Wv = np.asarray(inputs["Wv"], np.float32).reshape(L, D, N * H)
    Wo = np.asarray(inputs["Wo"], np.float32).reshape(L, N * H, D)
    Wf1 = np.asarray(inputs["Wf1"], np.float32)            # [L, D, FILT]
    bf1 = np.asarray(inputs["bf1"], np.float32)            # [L, FILT]
    Wf2 = np.asarray(inputs["Wf2"], np.float32)            # [L, FILT, D]
    bf2 = np.asarray(inputs["bf2"], np.float32)            # [L, D]

    # wf1 pre-tiled for lhsT streaming: [L, FK, DC, 128, 128]
    wf1_t = Wf1.reshape(L, DC, 128, FK, 128).transpose(0, 3, 1, 2, 4)

    consts = {
        "wq": Wq.astype(BF_NP),
        "wk": Wk.astype(BF_NP),
        "wv": Wv.astype(BF_NP),
        "wo": Wo.astype(BF_NP),
        "wf1t": np.ascontiguousarray(wf1_t).astype(BF_NP),
        "wf2": Wf2.astype(BF_NP),
        "bf1": bf1.reshape(L, FK, 128).astype(BF_NP),      # [L, FK, 128]
        "bf2": bf2.reshape(L, 1, D).astype(BF_NP),
        "idm": np.eye(128, dtype=np.float32).astype(BF_NP),
        "ones": np.ones((1, 128), np.float32).astype(BF_NP),
    }
    return consts


def _prep_call_inputs(inputs):
    """Per-call activation inputs (ExternalInput, small)."""
    di = np.asarray(inputs["decoder_inputs"], np.float32)
    eo = np.asarray(inputs["encoder_outputs"], np.float32)
    dist = np.asarray(inputs["decoder_encoder_times_dist"], np.float32)
    eb = np.asarray(inputs["enc_dec_attn_bias"], np.float32)
    Wth = np.asarray(inputs["Wth"], np.float32)
    bth = np.asarray(inputs["bth"], np.float32)
    Wto = np.asarray(inputs["Wto"], np.float32)
    bto = np.asarray(inputs["bto"], np.float32)

    # exact time-bias qs[i,b,f,t] = relu(d*Wth+bth) @ Wto + bto + eb[b,t]
    qs = np.empty((L, B, F, T), np.float32)
    for i in range(L):
        for f0 in range(0, F, 64):   # chunked: keep the [.,64,T,K] temp small
            h = np.maximum(dist[:, f0:f0 + 64, :, None] * Wth[i, 0] + bth[i],
                           0.0)
            qs[i, :, f0:f0 + 64] = h @ Wto[i, :, 0] + bto[i, 0]
    qs += eb[:, 0, 0][:, None, :][None]

    return {
        "x0": np.ascontiguousarray(di.reshape(ROWS, D)).astype(BF_NP),
        "encT": np.ascontiguousarray(eo.transpose(0, 2, 1)).astype(BF_NP),
        "qs": np.ascontiguousarray(qs.reshape(L, ROWS, T)).astype(BF_NP),
    }


def _weights_key(inputs):
    hsh = hashlib.sha256()
    for k in ("Wq", "Wk", "Wv", "Wo", "Wf1", "bf1", "Wf2", "bf2"):
        hsh.update(np.ascontiguousarray(np.asarray(inputs[k], np.float32)))
    return hsh.hexdigest()


# ------------------------------------------------------------ device program

def build_program(consts):
    nc = bacc.Bacc("TRN2", target_bir_lowering=False, debug=False,
                   num_devices=1)

    x0_d = nc.dram_tensor("x0", [ROWS, D], BF, kind="ExternalInput")
    encT_d = nc.dram_tensor("encT", [NB, D, T], BF, kind="ExternalInput")
    qs_d = nc.dram_tensor("qs", [L, ROWS, T], BF, kind="ExternalInput")
    yout_d = nc.dram_tensor("yout", [ROWS, D], BF, kind="ExternalOutput")

    wq_d = nc.inline_tensor(consts["wq"], name="wq")
    wk_d = nc.inline_tensor(consts["wk"], name="wk")
    wv_d = nc.inline_tensor(consts["wv"], name="wv")
    wo_d = nc.inline_tensor(consts["wo"], name="wo")
    wf1_d = nc.inline_tensor(consts["wf1t"], name="wf1t")
    wf2_d = nc.inline_tensor(consts["wf2"], name="wf2")
    bf1_d = nc.inline_tensor(consts["bf1"], name="bf1")
    bf2_d = nc.inline_tensor(consts["bf2"], name="bf2")
    id_d = nc.inline_tensor(consts["idm"], name="idm")
    ones_d = nc.inline_tensor(consts["ones"], name="ones")

    with tile.TileContext(nc) as tc, ExitStack() as ctx:
        per = ctx.enter_context(tc.tile_pool(name="per", bufs=1))
        kvp = ctx.enter_context(tc.tile_pool(name="kvp", bufs=1))
        wkv_p = ctx.enter_context(tc.tile_pool(name="wkv", bufs=2))
        wqo_p = ctx.enter_context(tc.tile_pool(name="wqo", bufs=1))
        lnp = ctx.enter_context(tc.tile_pool(name="lnp", bufs=2))
        blkp = ctx.enter_context(tc.tile_pool(name="blkp", bufs=1))
        qsp = ctx.enter_context(tc.tile_pool(name="qsp", bufs=2))
        smp = ctx.enter_context(tc.tile_pool(name="smp", bufs=2))
        otp = ctx.enter_context(tc.tile_pool(name="otp", bufs=2))
        htp = ctx.enter_context(tc.tile_pool(name="htp", bufs=1))
        wfp = ctx.enter_context(tc.tile_pool(name="wfp", bufs=4))
        psA = ctx.enter_context(tc.tile_pool(name="psA", bufs=2, space="PSUM"))
        psB = ctx.enter_context(tc.tile_pool(name="psB", bufs=2, space="PSUM"))
        psC = ctx.enter_context(tc.tile_pool(name="psC", bufs=2, space="PSUM"))
        psD = ctx.enter_context(tc.tile_pool(name="psD", bufs=2, space="PSUM"))
        ps_all = [psA, psB, psC, psD]

        def ps8():
            """Grab all 8 PSUM banks as 8 [128,512] fp32 tiles."""
            return [ps_all[j // 2].tile([128, 512], FP, tag="ABCD"[j // 2],
                                        name=f"ps8_{j}")
                    for j in range(8)]

        # ---- persistent tiles
        x_sb = per.tile([128, RT * D], FP)          # residual stream (rows)
        id_sb = per.tile([128, 128], BF)
        ones_sb = per.tile([1, 128], BF)
        enc_sb = per.tile([128, NB * DC * T], BF)   # encT both batches

        nc.sync.dma_start(id_sb[:], id_d[:, :])
        nc.sync.dma_start(ones_sb[:], ones_d[:, :])
        for r in range(RT):
            x0b = lnp.tile([128, D], BF, tag="xn")
            nc.sync.dma_start(x0b[:], x0_d[r * 128:(r + 1) * 128, :])
            nc.vector.tensor_copy(x_sb[:, r * D:(r + 1) * D], x0b[:])
        for nb in range(NB):
            for dc in range(DC):
                nc.sync.dma_start(
                    enc_sb[:, (nb * DC + dc) * T:(nb * DC + dc + 1) * T],
                    encT_d[nb, dc * 128:(dc + 1) * 128, :])

        def layer_norm(src_ap, dst_tile):
            """dst = (src - mean)/(std+eps); dst may be bf16."""
            scr = lnp.tile([128, D], FP, tag="scr", bufs=1)
            s1 = lnp.tile([128, 1], FP, tag="s1")
            nc.vector.tensor_reduce(s1[:], src_ap, AX.X, OP.add)
            sq = lnp.tile([128, 1], FP, tag="sq")
            nc.vector.scalar_tensor_tensor(scr[:], src_ap, 0.0, src_ap,
                                           OP.add, OP.mult, accum_out=sq[:])
            mean = lnp.tile([128, 1], FP, tag="mean")
            nc.scalar.mul(mean[:], s1[:], 1.0 / D)
            msq = lnp.tile([128, 1], FP, tag="msq")
            nc.vector.tensor_tensor(msq[:], mean[:], mean[:], OP.mult)
            var = lnp.tile([128, 1], FP, tag="var")
            nc.vector.scalar_tensor_tensor(var[:], sq[:], 1.0 / D, msq[:],
                                           OP.mult, OP.subtract)
            sd = lnp.tile([128, 1], FP, tag="sd")
            nc.scalar.activation(sd[:], var[:], AF.Sqrt)
            sde = lnp.tile([128, 1], FP, tag="sde")
            nc.vector.tensor_scalar_add(sde[:], sd[:], EPS)
            r_ = lnp.tile([128, 1], FP, tag="r")
            nc.vector.reciprocal(r_[:], sde[:])
            nb_ = lnp.tile([128, 1], FP, tag="nb")
            nc.vector.scalar_tensor_tensor(nb_[:], mean[:], -1.0, r_[:],
                                           OP.mult, OP.mult)
            nc.scalar.activation(dst_tile, src_ap, AF.Identity,
                                 bias=nb_[:, :1], scale=r_[:, :1])

        def transpose_rows(xn_tile, dstT, rt):
            """xn [128,D] bf16 -> dstT [128, DC*512] slices at col rt*128."""
            for g in range(2):          # two groups of 4 D-chunks
                pt = psD.tile([128, 512], BF, tag="D")
                for j in range(4):
                    dc = g * 4 + j
                    nc.tensor.transpose(pt[:, j * 128:(j + 1) * 128],
                                        xn_tile[:, dc * 128:(dc + 1) * 128],
                                        id_sb[:])
                dst = dstT[:, :].rearrange("p (dc r) -> p dc r", r=512)
                nc.vector.tensor_copy(
                    dst[:, g * 4:(g + 1) * 4, rt * 128:(rt + 1) * 128],
                    pt[:].rearrange("p (j r) -> p j r", j=4))
            return dstT

        # kT/v tiles for both batches (per layer)
        kT_sb = kvp.tile([128, NB * HC * T], BF, tag="kT")
        v_sb = kvp.tile([128, NB * 4 * N * H], BF, tag="v")

        def kv_proj(i):
            """K^T and V for both batches; wk/wv streamed per D-chunk with
            all 8 PSUM banks live as accumulators."""
            for nb in range(NB):
                kps = ps8()
                for dc in range(DC):
                    wk_sb = wkv_p.tile([128, N * H], BF, tag="wkv")
                    nc.sync.dma_start(wk_sb[:],
                                      wk_d[i, dc * 128:(dc + 1) * 128, :])
                    for hc in range(HC):
                        nc.tensor.matmul(
                            kps[hc][:],
                            wk_sb[:, hc * 128:(hc + 1) * 128],
                            enc_sb[:, (nb * DC + dc) * T:(nb * DC + dc + 1) * T],
                            start=(dc == 0), stop=(dc == DC - 1))
                for hc in range(HC):
                    nc.vector.tensor_copy(
                        kT_sb[:, (nb * HC + hc) * T:(nb * HC + hc + 1) * T],
                        kps[hc][:])
                vps = ps8()
                for dc in range(DC):
                    wv_sb = wkv_p.tile([128, N * H], BF, tag="wkv")
                    nc.sync.dma_start(wv_sb[:],
                                      wv_d[i, dc * 128:(dc + 1) * 128, :])
                    for tc in range(4):
                        for hh in range(2):
                            nc.tensor.matmul(
                                vps[tc * 2 + hh][:],
                                enc_sb[:, (nb * DC + dc) * T + tc * 128:
                                       (nb * DC + dc) * T + (tc + 1) * 128],
                                wv_sb[:, hh * 512:(hh + 1) * 512],
                                start=(dc == 0), stop=(dc == DC - 1))
                for tc in range(4):
                    for hh in range(2):
                        nc.vector.tensor_copy(
                            v_sb[:, (nb * 4 + tc) * N * H + hh * 512:
                                 (nb * 4 + tc) * N * H + (hh + 1) * 512],
                            vps[tc * 2 + hh][:])

        for i in range(L):
            # ---------------- K/V projections (both batches) ----------------
            kv_proj(i)

            # ---------------- attention ----------------
            wo_sb = wqo_p.tile([128, HC * D], BF, tag="wo")
            for hc in range(HC):
                nc.sync.dma_start(
                    wo_sb[:, hc * D:(hc + 1) * D],
                    wo_d[i, hc * 128:(hc + 1) * 128, :])

            for blk in range(RT // 4):        # 512-row blocks (one batch each)
                xnT = blkp.tile([128, DC * 512], BF, tag="xnT", bufs=2)
                for rt in range(4):
                    r = blk * 4 + rt
                    xn = lnp.tile([128, D], BF, tag="xn")
                    layer_norm(x_sb[:, r * D:(r + 1) * D], xn[:])
                    transpose_rows(xn, xnT, rt)

                # Q^T for the whole block: [h-chunk, 512 rows]; wq streamed
                qT = blkp.tile([128, HC * 512], BF, tag="qT", bufs=1)
                qps = ps8()
                for dc in range(DC):
                    wq_sb = wkv_p.tile([128, N * H], BF, tag="wkv")
                    nc.sync.dma_start(wq_sb[:],
                                      wq_d[i, dc * 128:(dc + 1) * 128, :])
                    for hc in range(HC):
                        nc.tensor.matmul(
                            qps[hc][:],
                            wq_sb[:, hc * 128:(hc + 1) * 128],
                            xnT[:, dc * 512:(dc + 1) * 512],
                            start=(dc == 0), stop=(dc == DC - 1))
                for hc in range(HC):
                    nc.vector.tensor_copy(qT[:, hc * 512:(hc + 1) * 512],
                                          qps[hc][:])

                for rt in range(4):
                    r = blk * 4 + rt
                    qs_sb = qsp.tile([128, T], BF, tag="qs")
                    nc.sync.dma_start(qs_sb[:],
                                      qs_d[i, r * 128:(r + 1) * 128, :])
                    qsf = qsp.tile([128, T], FP, tag="qsf")
                    nc.vector.tensor_copy(qsf[:], qs_sb[:])
                    oT_pss = [psC.tile([128, 512], FP, tag="C",
                                       name=f"oTps_{half}")
                              for half in range(2)]
                    for n in range(N):
                        hc, hr = n // 2, (n % 2) * 64
                        # 4-deep logits pipeline: alternate psA/psB banks
                        lg = ps_all[n % 2].tile([128, T], FP, tag="AB"[n % 2],
                                                name=f"lg{n % 2}")
                        nc.tensor.matmul(
                            lg[:],
                            qT[hr:hr + 64, hc * 512 + rt * 128:
                               hc * 512 + (rt + 1) * 128],
                            kT_sb[hr:hr + 64, (blk * HC + hc) * T:
                                  (blk * HC + hc + 1) * T],
                            start=True, stop=True)
                        # bias add in place in PSUM (GPSIMD cannot read PSUM)
                        nc.vector.tensor_tensor(lg[:], lg[:], qsf[:], OP.add)
                        den = lnp.tile([128, 1], FP, tag="den", bufs=4)
                        we = smp.tile([128, T], BF, tag="we", bufs=3)
                        nc.scalar.activation(we[:], lg[:], AF.Exp,
                                             accum_out=den[:])
                        rec = lnp.tile([128, 1], FP, tag="rec", bufs=4)
                        nc.vector.reciprocal(rec[:], den[:])
                        ws = smp.tile([128, T], BF, tag="ws", bufs=3)
                        if n % 2 == 0:
                            nc.vector.tensor_scalar_mul(ws[:], we[:],
                                                        rec[:, :1])
                        else:
                            nc.scalar.mul(ws[:], we[:], rec[:, :1])
                        # transpose w -> [t, f] chunks (psD is idle here)
                        pt = psD.tile([128, 4 * 128], BF, tag="D")
                        for tc in range(4):
                            nc.tensor.transpose(
                                pt[:, tc * 128:(tc + 1) * 128],
                                ws[:, tc * 128:(tc + 1) * 128], id_sb[:])
                        wT = smp.tile([128, 4 * 128], BF, tag="wT", bufs=3)
                        nc.vector.tensor_copy(wT[:], pt[:])
                        # AV: accumulate into oT psum rows for this head
                        ps_o = oT_pss[hc // 4]
                        for tc in range(4):
                            nc.tensor.matmul(
                                ps_o[hr:hr + 64,
                                     (hc % 4) * 128:(hc % 4 + 1) * 128],
                                v_sb[:, (blk * 4 + tc) * N * H + n * 64:
                                     (blk * 4 + tc) * N * H + (n + 1) * 64],
                                wT[:, tc * 128:(tc + 1) * 128],
                                start=(tc == 0), stop=(tc == 3))
                    oT = otp.tile([128, HC * 128], BF, tag="oT")
                    for half in range(2):
                        nc.vector.tensor_copy(
                            oT[:, half * 512:(half + 1) * 512],
                            oT_pss[half][:])
                    # O-projection + residual add (straight from PSUM)
                    for dh in range(2):
                        ps = psD.tile([128, 512], FP, tag="D")
                        for hc in range(HC):
                            nc.tensor.matmul(
                                ps[:],
                                oT[:, hc * 128:(hc + 1) * 128],
                                wo_sb[:, hc * D + dh * 512:
                                      hc * D + (dh + 1) * 512],
                                start=(hc == 0), stop=(hc == HC - 1))
                        xs = x_sb[:, r * D + dh * 512:r * D + (dh + 1) * 512]
                        nc.vector.tensor_tensor(xs, xs, ps[:], OP.add)

            # ---------------- FFN ----------------
            bf1_sb = lnp.tile([128, FK], BF, tag="bf1", bufs=1)
            nc.sync.dma_start(bf1_sb[:],
                              bf1_d[i].rearrange("fk p -> p fk"))
            bf2_sb = lnp.tile([1, D], BF, tag="bf2", bufs=1)
            nc.sync.dma_start(bf2_sb[:], bf2_d[i])

            for blk in range(RT // 4):
                xnT = blkp.tile([128, DC * 512], BF, tag="xnT", bufs=2)
                for rt in range(4):
                    r = blk * 4 + rt
                    xn = lnp.tile([128, D], BF, tag="xn")
                    layer_norm(x_sb[:, r * D:(r + 1) * D], xn[:])
                    transpose_rows(xn, xnT, rt)

                # FFN1: hT[filt-chunk, 512 rows] with fused bias+relu
                hT = htp.tile([128, FK * 512], BF, tag="hT")
                for fk in range(FK):
                    wf1_sb = wfp.tile([128, DC * 128], BF, tag="wf1")
                    nc.sync.dma_start(
                        wf1_sb[:].rearrange("p (dc c) -> p dc c", dc=DC),
                        wf1_d[i, fk].rearrange("dc p c -> p dc c"))
                    ps = ps_all[fk % 2].tile([128, 512], FP, tag="AB"[fk % 2])
                    for dc in range(DC):
                        nc.tensor.matmul(
                            ps[:],
                            wf1_sb[:, dc * 128:(dc + 1) * 128],
                            xnT[:, dc * 512:(dc + 1) * 512],
                            start=(dc == 0), stop=(dc == DC - 1))
                    nc.scalar.activation(hT[:, fk * 512:(fk + 1) * 512],
                                         ps[:], AF.Relu,
                                         bias=bf1_sb[:, fk:fk + 1])

                # FFN2: y2[f, D] accumulated over all filter chunks
                y2_ps = ps8()
                for fk in range(FK):
                    wf2_sb = wfp.tile([128, D], BF, tag="wf2")
                    nc.sync.dma_start(
                        wf2_sb[:], wf2_d[i, fk * 128:(fk + 1) * 128, :])
                    for rt in range(4):
                        for dh in range(2):
                            nc.tensor.matmul(
                                y2_ps[rt * 2 + dh][:],
                                hT[:, fk * 512 + rt * 128:
                                   fk * 512 + (rt + 1) * 128],
                                wf2_sb[:, dh * 512:(dh + 1) * 512],
                                start=(fk == 0), stop=False)
                # bias fold: y2 += ones^T @ bf2  (K=1 matmul, ends the group)
                for rt in range(4):
                    for dh in range(2):
                        nc.tensor.matmul(
                            y2_ps[rt * 2 + dh][:],
                            ones_sb[:, :],
                            bf2_sb[:, dh * 512:(dh + 1) * 512],
                            start=False, stop=True)
                for rt in range(4):
                    r = blk * 4 + rt
                    for dh in range(2):
                        xs = x_sb[:, r * D + dh * 512:r * D + (dh + 1) * 512]
                        nc.vector.tensor_tensor(xs, xs,
                                                y2_ps[rt * 2 + dh][:], OP.add)

        # final norm
        for r in range(RT):
            xf = lnp.tile([128, D], BF, tag="xfin")
            layer_norm(x_sb[:, r * D:(r + 1) * D], xf[:])
            nc.sync.dma_start(yout_d[r * 128:(r + 1) * 128, :], xf[:])

    nc.compile()
    return nc


# ------------------------------------------------------------ runner

_CACHE = {}   # weights_key -> (sharded, in_names, zero_outs)


def _build_runner(consts):
    import jax
    from jax.sharding import Mesh, PartitionSpec
    from jax.experimental.shard_map import shard_map
    from concourse import bass2jax

    nc = build_program(consts)
    partition_name = (nc.partition_id_tensor.name
                      if nc.partition_id_tensor else None)
    in_names, out_names, out_avals = [], [], []
    for alloc in nc.m.functions[0].allocations:
        if not isinstance(alloc, mybir.MemoryLocationSet):
            continue
        name = alloc.memorylocations[0].name
        if alloc.kind == "ExternalInput":
            if name != partition_name:
                in_names.append(name)
        elif alloc.kind == "ExternalOutput":
            out_names.append(name)
            out_avals.append(jax.core.ShapedArray(
                tuple(alloc.tensor_shape), mybir.dt.np(alloc.dtype)))
    all_names = in_names + out_names
    if partition_name is not None:
        all_names = all_names + [partition_name]

    def _body(*args):
        operands = list(args)
        if partition_name is not None:
            operands.append(bass2jax.partition_id_tensor())
        outs = bass2jax._bass_exec_p.bind(
            *operands,
            out_avals=tuple(out_avals),
            in_names=tuple(all_names),
            out_names=tuple(out_names),
            lowering_input_output_aliases=(),
            sim_require_finite=True,
            sim_require_nnan=True,
            nc=nc,
        )
        return tuple(outs)

    bass2jax.install_neuronx_cc_hook()
    devices = jax.devices()[:1]
    mesh = Mesh(np.asarray(devices), ("core",))
    n_all = len(in_names) + len(out_names)
    sharded = jax.jit(
        shard_map(_body, mesh=mesh,
                  in_specs=(PartitionSpec("core"),) * n_all,
                  out_specs=(PartitionSpec("core"),) * len(out_names),
                  check_rep=False),
        keep_unused=True,
    )
    zero_outs = [np.zeros(tuple(a.shape), a.dtype) for a in out_avals]
    return sharded, in_names, out_names, zero_outs


def _get_runner(inputs):
    key = _weights_key(inputs)
    if key not in _CACHE:
        consts = _prep_consts(inputs)
        _CACHE[key] = _build_runner(consts)
    return _CACHE[key]


def kernel(**inputs) -> np.ndarray:
    sharded, in_names, out_names, zero_outs = _get_runner(inputs)
    call_in = _prep_call_inputs(inputs)
    args = [call_in[nm] for nm in in_names] + zero_outs
    out_arrs = sharded(*args)
    yi = out_names.index("yout")
    yfull = np.asarray(out_arrs[yi]).reshape(B, F, D)
    return yfull


if __name__ == "__main__":
    import sys
    sys.path.insert(0, "/root/problem")
    import reference
    inputs = {k: np.asarray(v) for k, v in reference.setup_inputs().items()}
    expected = np.asarray(reference.reference(**inputs))
    got = kernel(**inputs)
    err = np.abs(got - expected).max() / np.abs(expected).max()
    print("rel err (absmax):", err)
    print("rel l2:", np.linalg.norm(got - expected) / np.linalg.norm(expected))


# revision 32
# speedup vs baseline: 1.4657x; 1.0839x over previous
"""Trainium2 Bass kernel for nn_DecoderStack (cross-attention decoder stack).

Strategy (v2): single NeuronCore, zero collectives, weights baked into the
NEFF as bf16 constants.

Why: on this axon-tunneled platform the per-call cost is dominated by
fixed dispatch overhead (~4-5 ms/device-mesh) plus re-staging of every
ExternalInput buffer (~11 GB/s). The old 8-core tensor-parallel kernel
shipped 341 MB of fp32 weights per call => ~40 ms/call. Device compute for
the whole model is only ~1.5 ms on one core in bf16. So:
  - run on ONE core (smallest dispatch floor),
  - bake all weights into the NEFF via inline_tensor (Const tensors are
    loaded to HBM once at model-load, not per call),
  - per-call ExternalInputs are just the activations (~8 MB, all bf16):
    x0, encoder^T, and the precomputed time-bias qs.

The decoder rows are fully independent (enc-dec cross attention only; the
self-attn inputs are unused by the reference), so all B*F=1024 rows stream
through the stack with no inter-row dependencies. All matmuls run in bf16
(fp32 PSUM accumulate); layernorm/softmax statistics stay fp32.

kernel() re-builds (and re-compiles) the program if called with weights
whose bytes differ from the baked ones - correctness never depends on the
cache, only speed.
"""
import hashlib
import numpy as np
from contextlib import ExitStack

import concourse.bass as bass
import concourse.bacc as bacc
import concourse.tile as tile
from concourse import mybir

B, F, T = 2, 512, 512
D, N, H = 1024, 16, 64
KSZ = 32
FILT = 4096
L = 4
EPS = 1e-6

ROWS = B * F          # 1024 decoder rows total
RT = ROWS // 128      # 8 row-tiles
NB = B                # batches on this core
DC = D // 128         # 8 contraction chunks
FK = FILT // 128      # 32 filter chunks
HC = N * H // 128     # 8 head-dim chunks (2 heads per chunk)

FP = mybir.dt.float32
BF = mybir.dt.bfloat16
AF = mybir.ActivationFunctionType
OP = mybir.AluOpType
AX = mybir.AxisListType

BF_NP = mybir.dt.np(BF)


# ---------------------------------------------------------------- host prep

def _prep_consts(inputs):
    """Weight-derived constant arrays (baked into the NEFF, bf16)."""
    Wq = (np.asarray(inputs["Wq"], np.float32) * np.float32(H ** -0.5)
          ).reshape(L, D, N * H)
    Wk = np.asarray(inputs["Wk"], np.float32).reshape(L, D, N * H)
    Wv = np.asarray(inputs["Wv"], np.float32).reshape(L, D, N * H)
    Wo = np.asarray(inputs["Wo"], np.float32).reshape(L, N * H, D)
    Wf1 = np.asarray(inputs["Wf1"], np.float32)            # [L, D, FILT]
    bf1 = np.asarray(inputs["bf1"], np.float32)            # [L, FILT]
    Wf2 = np.asarray(inputs["Wf2"], np.float32)            # [L, FILT, D]
    bf2 = np.asarray(inputs["bf2"], np.float32)            # [L, D]

    # wf1 pre-tiled for lhsT streaming: [L, FK, 128, DC, 128]
    wf1_t = Wf1.reshape(L, DC, 128, FK, 128).transpose(0, 3, 2, 1, 4)

    consts = {
        "wq": Wq.astype(BF_NP),
        "wk": Wk.astype(BF_NP),
        "wv": Wv.astype(BF_NP),
        "wo": Wo.astype(BF_NP),
        "wf1t": np.ascontiguousarray(wf1_t).astype(BF_NP),
        "wf2": Wf2.astype(BF_NP),
        "bf1": bf1.reshape(L, FK, 128).astype(BF_NP),      # [L, FK, 128]
        "bf2": bf2.reshape(L, 1, D).astype(BF_NP),
        "idm": np.eye(128, dtype=np.float32).astype(BF_NP),
        "ones": np.ones((1, 128), np.float32).astype(BF_NP),
    }
    return consts


def _prep_call_inputs(inputs):
    """Per-call activation inputs (ExternalInput, small)."""
    di = np.asarray(inputs["decoder_inputs"], np.float32)
    eo = np.asarray(inputs["encoder_outputs"], np.float32)
    dist = np.asarray(inputs["decoder_encoder_times_dist"], np.float32)
    eb = np.asarray(inputs["enc_dec_attn_bias"], np.float32)
    Wth = np.asarray(inputs["Wth"], np.float32)
    bth = np.asarray(inputs["bth"], np.float32)
    Wto = np.asarray(inputs["Wto"], np.float32)
    bto = np.asarray(inputs["bto"], np.float32)

    # exact time-bias qs[i,b,f,t] = relu(d*Wth+bth) @ Wto + bto + eb[b,t]
    qs = np.empty((L, B, F, T), np.float32)
    for i in range(L):
        for f0 in range(0, F, 64):   # chunked: keep the [.,64,T,K] temp small
            h = np.maximum(dist[:, f0:f0 + 64, :, None] * Wth[i, 0] + bth[i],
                           0.0)
            qs[i, :, f0:f0 + 64] = h @ Wto[i, :, 0] + bto[i, 0]
    qs += eb[:, 0, 0][:, None, :][None]

    return {
        "x0": np.ascontiguousarray(di.reshape(ROWS, D)).astype(BF_NP),
        "encT": np.ascontiguousarray(eo.transpose(0, 2, 1)).astype(BF_NP),
        "qs": np.ascontiguousarray(qs.reshape(L, ROWS, T)).astype(BF_NP),
    }


def _weights_key(inputs):
    hsh = hashlib.sha256()
    for k in ("Wq", "Wk", "Wv", "Wo", "Wf1", "bf1", "Wf2", "bf2"):
        hsh.update(np.ascontiguousarray(np.asarray(inputs[k], np.float32)))
    return hsh.hexdigest()


# ------------------------------------------------------------ device program

def build_program(consts):
    nc = bacc.Bacc("TRN2", target_bir_lowering=False, debug=False,
                   num_devices=1)

    x0_d = nc.dram_tensor("x0", [ROWS, D], BF, kind="ExternalInput")
    encT_d = nc.dram_tensor("encT", [NB, D, T], BF, kind="ExternalInput")
    qs_d = nc.dram_tensor("qs", [L, ROWS, T], BF, kind="ExternalInput")
    yout_d = nc.dram_tensor("yout", [ROWS, D], BF, kind="ExternalOutput")

    wq_d = nc.inline_tensor(consts["wq"], name="wq")
    wk_d = nc.inline_tensor(consts["wk"], name="wk")
    wv_d = nc.inline_tensor(consts["wv"], name="wv")
    wo_d = nc.inline_tensor(consts["wo"], name="wo")
    wf1_d = nc.inline_tensor(consts["wf1t"], name="wf1t")
    wf2_d = nc.inline_tensor(consts["wf2"], name="wf2")
    bf1_d = nc.inline_tensor(consts["bf1"], name="bf1")
    bf2_d = nc.inline_tensor(consts["bf2"], name="bf2")
    id_d = nc.inline_tensor(consts["idm"], name="idm")
    ones_d = nc.inline_tensor(consts["ones"], name="ones")

    with tile.TileContext(nc) as tc, ExitStack() as ctx:
        per = ctx.enter_context(tc.tile_pool(name="per", bufs=1))
        kvp = ctx.enter_context(tc.tile_pool(name="kvp", bufs=1))
        wkv_p = ctx.enter_context(tc.tile_pool(name="wkv", bufs=2))
        wqo_p = ctx.enter_context(tc.tile_pool(name="wqo", bufs=1))
        lnp = ctx.enter_context(tc.tile_pool(name="lnp", bufs=2))
        blkp = ctx.enter_context(tc.tile_pool(name="blkp", bufs=1))
        qsp = ctx.enter_context(tc.tile_pool(name="qsp", bufs=2))
        smp = ctx.enter_context(tc.tile_pool(name="smp", bufs=2))
        otp = ctx.enter_context(tc.tile_pool(name="otp", bufs=2))
        htp = ctx.enter_context(tc.tile_pool(name="htp", bufs=1))
        wfp = ctx.enter_context(tc.tile_pool(name="wfp", bufs=4))
        psA = ctx.enter_context(tc.tile_pool(name="psA", bufs=2, space="PSUM"))
        psB = ctx.enter_context(tc.tile_pool(name="psB", bufs=2, space="PSUM"))
        psC = ctx.enter_context(tc.tile_pool(name="psC", bufs=2, space="PSUM"))
        psD = ctx.enter_context(tc.tile_pool(name="psD", bufs=2, space="PSUM"))
        ps_all = [psA, psB, psC, psD]

        def ps8():
            """Grab all 8 PSUM banks as 8 [128,512] fp32 tiles."""
            return [ps_all[j // 2].tile([128, 512], FP, tag="ABCD"[j // 2],
                                        name=f"ps8_{j}")
                    for j in range(8)]

        # ---- persistent tiles
        x_sb = per.tile([128, RT * D], FP)          # residual stream (rows)
        id_sb = per.tile([128, 128], BF)
        ones_sb = per.tile([1, 128], BF)
        enc_sb = per.tile([128, NB * DC * T], BF)   # encT both batches

        nc.sync.dma_start(id_sb[:], id_d[:, :])
        nc.sync.dma_start(ones_sb[:], ones_d[:, :])
        for r in range(RT):
            x0b = lnp.tile([128, D], BF, tag="xn")
            nc.sync.dma_start(x0b[:], x0_d[r * 128:(r + 1) * 128, :])
            nc.vector.tensor_copy(x_sb[:, r * D:(r + 1) * D], x0b[:])
        for nb in range(NB):
            for dc in range(DC):
                nc.sync.dma_start(
                    enc_sb[:, (nb * DC + dc) * T:(nb * DC + dc + 1) * T],
                    encT_d[nb, dc * 128:(dc + 1) * 128, :])

        def layer_norm(src_ap, dst_tile):
            """dst = (src - mean)/(std+eps); dst may be bf16."""
            scr = lnp.tile([128, D], FP, tag="scr", bufs=1)
            s1 = lnp.tile([128, 1], FP, tag="s1")
            nc.vector.tensor_reduce(s1[:], src_ap, AX.X, OP.add)
            sq = lnp.tile([128, 1], FP, tag="sq")
            nc.vector.scalar_tensor_tensor(scr[:], src_ap, 0.0, src_ap,
                                           OP.add, OP.mult, accum_out=sq[:])
            mean = lnp.tile([128, 1], FP, tag="mean")
            nc.scalar.mul(mean[:], s1[:], 1.0 / D)
            msq = lnp.tile([128, 1], FP, tag="msq")
            nc.vector.tensor_tensor(msq[:], mean[:], mean[:], OP.mult)
            var = lnp.tile([128, 1], FP, tag="var")
            nc.vector.scalar_tensor_tensor(var[:], sq[:], 1.0 / D, msq[:],
                                           OP.mult, OP.subtract)
            sd = lnp.tile([128, 1], FP, tag="sd")
            nc.scalar.activation(sd[:], var[:], AF.Sqrt)
            sde = lnp.tile([128, 1], FP, tag="sde")
            nc.vector.tensor_scalar_add(sde[:], sd[:], EPS)
            r_ = lnp.tile([128, 1], FP, tag="r")
            nc.vector.reciprocal(r_[:], sde[:])
            nb_ = lnp.tile([128, 1], FP, tag="nb")
            nc.vector.scalar_tensor_tensor(nb_[:], mean[:], -1.0, r_[:],
                                           OP.mult, OP.mult)
            nc.scalar.activation(dst_tile, src_ap, AF.Identity,
                                 bias=nb_[:, :1], scale=r_[:, :1])

        def transpose_rows(xn_tile, dstT, rt):
            """xn [128,D] bf16 -> dstT [128, DC*512] slices at col rt*128."""
            for g in range(2):          # two groups of 4 D-chunks
                pt = psD.tile([128, 512], BF, tag="D")
                for j in range(4):
                    dc = g * 4 + j
                    nc.tensor.transpose(pt[:, j * 128:(j + 1) * 128],
                                        xn_tile[:, dc * 128:(dc + 1) * 128],
                                        id_sb[:])
                dst = dstT[:, :].rearrange("p (dc r) -> p dc r", r=512)
                nc.vector.tensor_copy(
                    dst[:, g * 4:(g + 1) * 4, rt * 128:(rt + 1) * 128],
                    pt[:].rearrange("p (j r) -> p j r", j=4))
            return dstT

        # kT/v tiles for both batches (per layer)
        kT_sb = kvp.tile([128, NB * HC * T], BF, tag="kT")
        v_sb = kvp.tile([128, NB * 4 * N * H], BF, tag="v")

        def kv_proj(i):
            """K^T and V for both batches; wk/wv streamed per D-chunk with
            all 8 PSUM banks live as accumulators."""
            for nb in range(NB):
                kps = ps8()
                for dc in range(DC):
                    wk_sb = wkv_p.tile([128, N * H], BF, tag="wkv")
                    nc.sync.dma_start(wk_sb[:],
                                      wk_d[i, dc * 128:(dc + 1) * 128, :])
                    for hc in range(HC):
                        nc.tensor.matmul(
                            kps[hc][:],
                            wk_sb[:, hc * 128:(hc + 1) * 128],
                            enc_sb[:, (nb * DC + dc) * T:(nb * DC + dc + 1) * T],
                            start=(dc == 0), stop=(dc == DC - 1))
                for hc in range(HC):
                    nc.vector.tensor_copy(
                        kT_sb[:, (nb * HC + hc) * T:(nb * HC + hc + 1) * T],
                        kps[hc][:])
                vps = ps8()
                for dc in range(DC):
                    wv_sb = wkv_p.tile([128, N * H], BF, tag="wkv")
                    nc.sync.dma_start(wv_sb[:],
                                      wv_d[i, dc * 128:(dc + 1) * 128, :])
                    for tc in range(4):
                        for hh in range(2):
                            nc.tensor.matmul(
                                vps[tc * 2 + hh][:],
                                enc_sb[:, (nb * DC + dc) * T + tc * 128:
                                       (nb * DC + dc) * T + (tc + 1) * 128],
                                wv_sb[:, hh * 512:(hh + 1) * 512],
                                start=(dc == 0), stop=(dc == DC - 1))
                for tc in range(4):
                    for hh in range(2):
                        nc.vector.tensor_copy(
                            v_sb[:, (nb * 4 + tc) * N * H + hh * 512:
                                 (nb * 4 + tc) * N * H + (hh + 1) * 512],
                            vps[tc * 2 + hh][:])

        for i in range(L):
            # ---------------- K/V projections (both batches) ----------------
            kv_proj(i)

            # ---------------- attention ----------------
            wo_sb = wqo_p.tile([128, HC * D], BF, tag="wo")
            for hc in range(HC):
                nc.sync.dma_start(
                    wo_sb[:, hc * D:(hc + 1) * D],
                    wo_d[i, hc * 128:(hc + 1) * 128, :])

            for blk in range(RT // 4):        # 512-row blocks (one batch each)
                xnT = blkp.tile([128, DC * 512], BF, tag="xnT", bufs=2)
                for rt in range(4):
                    r = blk * 4 + rt
                    xn = lnp.tile([128, D], BF, tag="xn")
                    layer_norm(x_sb[:, r * D:(r + 1) * D], xn[:])
                    transpose_rows(xn, xnT, rt)

                # Q^T for the whole block: [h-chunk, 512 rows]; wq streamed
                qT = blkp.tile([128, HC * 512], BF, tag="qT", bufs=1)
                qps = ps8()
                for dc in range(DC):
                    wq_sb = wkv_p.tile([128, N * H], BF, tag="wkv")
                    nc.sync.dma_start(wq_sb[:],
                                      wq_d[i, dc * 128:(dc + 1) * 128, :])
                    for hc in range(HC):
                        nc.tensor.matmul(
                            qps[hc][:],
                            wq_sb[:, hc * 128:(hc + 1) * 128],
                            xnT[:, dc * 512:(dc + 1) * 512],
                            start=(dc == 0), stop=(dc == DC - 1))
                for hc in range(HC):
                    nc.vector.tensor_copy(qT[:, hc * 512:(hc + 1) * 512],
                                          qps[hc][:])

                for rt in range(4):
                    r = blk * 4 + rt
                    qs_sb = qsp.tile([128, T], BF, tag="qs")
                    nc.sync.dma_start(qs_sb[:],
                                      qs_d[i, r * 128:(r + 1) * 128, :])
                    qsf = qsp.tile([128, T], FP, tag="qsf")
                    nc.vector.tensor_copy(qsf[:], qs_sb[:])
                    oT_pss = [psC.tile([128, 512], FP, tag="C",
                                       name=f"oTps_{half}")
                              for half in range(2)]
                    for n in range(N):
                        hc, hr = n // 2, (n % 2) * 64
                        # 4-deep logits pipeline: alternate psA/psB banks
                        lg = ps_all[n % 2].tile([128, T], FP, tag="AB"[n % 2],
                                                name=f"lg{n % 2}")
                        nc.tensor.matmul(
                            lg[:],
                            qT[hr:hr + 64, hc * 512 + rt * 128:
                               hc * 512 + (rt + 1) * 128],
                            kT_sb[hr:hr + 64, (blk * HC + hc) * T:
                                  (blk * HC + hc + 1) * T],
                            start=True, stop=True)
                        # bias add in place in PSUM (GPSIMD cannot read PSUM)
                        nc.vector.tensor_tensor(lg[:], lg[:], qsf[:], OP.add)
                        den = lnp.tile([128, 1], FP, tag="den", bufs=4)
                        we = smp.tile([128, T], BF, tag="we", bufs=3)
                        nc.scalar.activation(we[:], lg[:], AF.Exp,
                                             accum_out=den[:])
                        rec = lnp.tile([128, 1], FP, tag="rec", bufs=4)
                        nc.vector.reciprocal(rec[:], den[:])
                        ws = smp.tile([128, T], BF, tag="ws", bufs=3)
                        if n % 2 == 0:
                            nc.vector.tensor_scalar_mul(ws[:], we[:],
                                                        rec[:, :1])
                        else:
                            nc.scalar.mul(ws[:], we[:], rec[:, :1])
                        # transpose w -> [t, f] chunks (psD is idle here)
                        pt = psD.tile([128, 4 * 128], BF, tag="D")
                        for tc in range(4):
                            nc.tensor.transpose(
                                pt[:, tc * 128:(tc + 1) * 128],
                                ws[:, tc * 128:(tc + 1) * 128], id_sb[:])
                        wT = smp.tile([128, 4 * 128], BF, tag="wT", bufs=3)
                        nc.vector.tensor_copy(wT[:], pt[:])
                        # AV: accumulate into oT psum rows for this head
                        ps_o = oT_pss[hc // 4]
                        for tc in range(4):
                            nc.tensor.matmul(
                                ps_o[hr:hr + 64,
                                     (hc % 4) * 128:(hc % 4 + 1) * 128],
                                v_sb[:, (blk * 4 + tc) * N * H + n * 64:
                                     (blk * 4 + tc) * N * H + (n + 1) * 64],
                                wT[:, tc * 128:(tc + 1) * 128],
                                start=(tc == 0), stop=(tc == 3))
                    oT = otp.tile([128, HC * 128], BF, tag="oT")
                    for half in range(2):
                        nc.vector.tensor_copy(
                            oT[:, half * 512:(half + 1) * 512],
                            oT_pss[half][:])
                    # O-projection + residual add (straight from PSUM)
                    for dh in range(2):
                        ps = psD.tile([128, 512], FP, tag="D")
                        for hc in range(HC):
                            nc.tensor.matmul(
                                ps[:],
                                oT[:, hc * 128:(hc + 1) * 128],
                                wo_sb[:, hc * D + dh * 512:
                                      hc * D + (dh + 1) * 512],
                                start=(hc == 0), stop=(hc == HC - 1))
                        xs = x_sb[:, r * D + dh * 512:r * D + (dh + 1) * 512]
                        nc.vector.tensor_tensor(xs, xs, ps[:], OP.add)

            # ---------------- FFN ----------------
            bf1_sb = lnp.tile([128, FK], BF, tag="bf1", bufs=1)
            nc.sync.dma_start(bf1_sb[:],
                              bf1_d[i].rearrange("fk p -> p fk"))
            bf2_sb = lnp.tile([1, D], BF, tag="bf2", bufs=1)
            nc.sync.dma_start(bf2_sb[:], bf2_d[i])

            for blk in range(RT // 4):
                xnT = blkp.tile([128, DC * 512], BF, tag="xnT", bufs=2)
                for rt in range(4):
                    r = blk * 4 + rt
                    xn = lnp.tile([128, D], BF, tag="xn")
                    layer_norm(x_sb[:, r * D:(r + 1) * D], xn[:])
                    transpose_rows(xn, xnT, rt)

                # FFN1: hT[filt-chunk, 512 rows] with fused bias+relu
                hT = htp.tile([128, FK * 512], BF, tag="hT")
                for fk in range(FK):
                    wf1_sb = wfp.tile([128, DC * 128], BF, tag="wf1")
                    nc.sync.dma_start(
                        wf1_sb[:].rearrange("p (dc c) -> p dc c", dc=DC),
                        wf1_d[i, fk])
                    ps = ps_all[fk % 2].tile([128, 512], FP, tag="AB"[fk % 2])
                    for dc in range(DC):
                        nc.tensor.matmul(
                            ps[:],
                            wf1_sb[:, dc * 128:(dc + 1) * 128],
                            xnT[:, dc * 512:(dc + 1) * 512],
                            start=(dc == 0), stop=(dc == DC - 1))
                    nc.scalar.activation(hT[:, fk * 512:(fk + 1) * 512],
                                         ps[:], AF.Relu,
                                         bias=bf1_sb[:, fk:fk + 1])

                # FFN2: y2[f, D] accumulated over all filter chunks
                y2_ps = ps8()
                for fk in range(FK):
                    wf2_sb = wfp.tile([128, D], BF, tag="wf2")
                    nc.sync.dma_start(
                        wf2_sb[:], wf2_d[i, fk * 128:(fk + 1) * 128, :])
                    for rt in range(4):
                        for dh in range(2):
                            nc.tensor.matmul(
                                y2_ps[rt * 2 + dh][:],
                                hT[:, fk * 512 + rt * 128:
                                   fk * 512 + (rt + 1) * 128],
                                wf2_sb[:, dh * 512:(dh + 1) * 512],
                                start=(fk == 0), stop=False)
                # bias fold: y2 += ones^T @ bf2  (K=1 matmul, ends the group)
                for rt in range(4):
                    for dh in range(2):
                        nc.tensor.matmul(
                            y2_ps[rt * 2 + dh][:],
                            ones_sb[:, :],
                            bf2_sb[:, dh * 512:(dh + 1) * 512],
                            start=False, stop=True)
                for rt in range(4):
                    r = blk * 4 + rt
                    for dh in range(2):
                        xs = x_sb[:, r * D + dh * 512:r * D + (dh + 1) * 512]
                        nc.vector.tensor_tensor(xs, xs,
                                                y2_ps[rt * 2 + dh][:], OP.add)

        # final norm
        for r in range(RT):
            xf = lnp.tile([128, D], BF, tag="xfin")
            layer_norm(x_sb[:, r * D:(r + 1) * D], xf[:])
            nc.sync.dma_start(yout_d[r * 128:(r + 1) * 128, :], xf[:])

    nc.compile()
    return nc


# ------------------------------------------------------------ runner

_CACHE = {}   # weights_key -> (sharded, in_names, zero_outs)


def _build_runner(consts):
    import jax
    from jax.sharding import Mesh, PartitionSpec
    from jax.experimental.shard_map import shard_map
    from concourse import bass2jax

    nc = build_program(consts)
    partition_name = (nc.partition_id_tensor.name
                      if nc.partition_id_tensor else None)
    in_names, out_names, out_avals = [], [], []
    for alloc in nc.m.functions[0].allocations:
        if not isinstance(alloc, mybir.MemoryLocationSet):
            continue
        name = alloc.memorylocations[0].name
        if alloc.kind == "ExternalInput":
            if name != partition_name:
                in_names.append(name)
        elif alloc.kind == "ExternalOutput":
            out_names.append(name)
            out_avals.append(jax.core.ShapedArray(
                tuple(alloc.tensor_shape), mybir.dt.np(alloc.dtype)))
    all_names = in_names + out_names
    if partition_name is not None:
        all_names = all_names + [partition_name]

    def _body(*args):
        operands = list(args)
        if partition_name is not None:
            operands.append(bass2jax.partition_id_tensor())
        outs = bass2jax._bass_exec_p.bind(
            *operands,
            out_avals=tuple(out_avals),
            in_names=tuple(all_names),
            out_names=tuple(out_names),
            lowering_input_output_aliases=(),
            sim_require_finite=True,
            sim_require_nnan=True,
            nc=nc,
        )
        return tuple(outs)

    bass2jax.install_neuronx_cc_hook()
    devices = jax.devices()[:1]
    mesh = Mesh(np.asarray(devices), ("core",))
    n_all = len(in_names) + len(out_names)
    sharded = jax.jit(
        shard_map(_body, mesh=mesh,
                  in_specs=(PartitionSpec("core"),) * n_all,
                  out_specs=(PartitionSpec("core"),) * len(out_names),
                  check_rep=False),
        keep_unused=True,
    )
    zero_outs = [np.zeros(tuple(a.shape), a.dtype) for a in out_avals]
    return sharded, in_names, out_names, zero_outs


def _get_runner(inputs):
    key = _weights_key(inputs)
    if key not in _CACHE:
        consts = _prep_consts(inputs)
        _CACHE[key] = _build_runner(consts)
    return _CACHE[key]


def kernel(**inputs) -> np.ndarray:
    sharded, in_names, out_names, zero_outs = _get_runner(inputs)
    call_in = _prep_call_inputs(inputs)
    args = [call_in[nm] for nm in in_names] + zero_outs
    out_arrs = sharded(*args)
    yi = out_names.index("yout")
    yfull = np.asarray(out_arrs[yi]).reshape(B, F, D)
    return yfull


if __name__ == "__main__":
    import sys
    sys.path.insert(0, "/root/problem")
    import reference
    inputs = {k: np.asarray(v) for k, v in reference.setup_inputs().items()}
    expected = np.asarray(reference.reference(**inputs))
    got = kernel(**inputs)
    err = np.abs(got - expected).max() / np.abs(expected).max()
    print("rel err (absmax):", err)
    print("rel l2:", np.linalg.norm(got - expected) / np.linalg.norm(expected))
